# revision 96
# baseline (speedup 1.0000x reference)
# kernel.py — DiscriminativeLoss on 8 TRN2 NeuronCores (Bass/Tile, SPMD).
#
# Math (matches reference):
#   counts_k = #{i: l_i = k};  S_k = sum_{i in k} x_i;  mu_k = S_k / max(c_k, 1)
#   intra = (1/K) * sum_i invc_{l_i} * relu(||x_i - mu_{l_i} + eps|| - 1.5)^2
#   inter = sum_{a != b} relu(1 - ||(mu_a + eps) - mu_b||)^2 / (K*(K-1))
#   reg   = (1/K) * sum_k ||mu_k + eps||
#   total = intra + inter + 0.001 * reg
#
# V2 design (engine-balanced, cost-model driven):
#   pass 1: one-hot H2 [P, K, jn] via DVE/Pool tensor_tensor is_equal (2x
#     mode: all operands 2-byte packed SBUF); PE matmul lhsT=Xe [128,33],
#     rhs=H2[:, :, j] accumulates S^T = [S | counts] in PSUM [33, 64].
#   AllReduce [33, 64]; stats (mu, 1/c, sqrt(1/c), inter/reg losses) on
#     Act/Pool/PE only, keeping DVE free.
#   pass 2: paired transposed one-hot ht [128, 128] per tile-pair (A on
#     partitions 0-63, B on 64-127) built from broadcast-DMA'd labels via
#     DVE tensor_single_scalar is_equal (4x mode). Per tile, TWO accumulating
#     matmuls produce diff = x - (mu - eps) directly in PSUM:
#       psD  = ht_half^T @ [eps - mu | sqrt(1/c) - 1]   (gather, negated)
#       psD += I_128    @ Xe_tile                        ([x | 1])
#     -> psD = [x - mu + eps | sqrt(1/c)].
#   Act Square psD -> sq bf16 (col 32 squares to 1/c); DVE/Pool tensor_reduce
#     over D -> d2; DVE copies col 32 -> invc_all.
#   finals: dist=sqrt(d2), h=relu(dist-1.5) on Act; intra partial
#     sum_i h^2 * invc via DVE mults + tensor_tensor_reduce + PE ones-matmul.
import math
import numpy as np
from contextlib import ExitStack

import concourse.bass as bass
import concourse.bacc as bacc
import concourse.tile as tile
import concourse.mybir as mybir
from concourse.bass_utils import run_bass_kernel_spmd

F32 = mybir.dt.float32
BF16 = mybir.dt.bfloat16
I16 = mybir.dt.int16

N_CORES = 8
K = 64
D = 32
P = 128
EPS = 1e-8
PAD_LABEL = 999  # never matches any one-hot column

INTRA_MARGIN = 1.5
INTER_MARGIN2 = 1.0  # 2 * 0.5

J1 = 20      # pass-1 chunk width (tiles)
NACT = 0     # pass-1 tiles whose one-hot is built on Act (PE rank-3 + relu)
TPAIR = 28   # tile-pairs per ht chunk (4 gather groups of 7 pairs)
JMG = 14     # tiles per PSUM gather group (7 pairs)


def _host_prep(features, labels, tpc):
    """Shard + relayout on host. Returns per-core input dicts."""
    n_total = features.shape[0]
    n_core = n_total // N_CORES
    n_pad = P * tpc
    npair = (tpc + 1) // 2
    import ml_dtypes

    in_maps = []
    for c in range(N_CORES):
        f = np.asarray(features[c * n_core : (c + 1) * n_core], dtype=np.float32)
        l = np.asarray(labels[c * n_core : (c + 1) * n_core], dtype=np.int64)
        if n_pad > n_core:
            f = np.concatenate([f, np.zeros((n_pad - n_core, D), np.float32)], axis=0)
            l = np.concatenate([l, np.full((n_pad - n_core,), PAD_LABEL, np.int64)])
        # Xe: [P, tpc, 33] bf16, col 32 = 1/256 (exact in bf16; keeps the
        # sqrt(1/c) gather free of bf16 cancellation); point i = (i%P, i//P)
        xe = np.full((n_pad, D + 1), 1.0 / 256.0, np.float32)
        xe[:, :D] = f
        xe = xe.reshape(P, tpc, D + 1).astype(ml_dtypes.bfloat16)
        lpj = l.reshape(P, tpc)  # [point-in-tile, tile]
        l_pm = lpj.astype(np.int16)
        # paired label broadcast for ht, fully materialized on host:
        # [128, npair*128] int16, rows 0-63 = labels of tile 2jj, rows
        # 64-127 = labels of tile 2jj+1 (one contiguous DMA per chunk)
        ltm = lpj.T.astype(np.int16)  # [tpc, P]
        l_tma = np.full((npair, P), PAD_LABEL, np.int16)
        l_tmb = np.full((npair, P), PAD_LABEL, np.int16)
        l_tma[:] = ltm[0::2]
        nb = tpc // 2
        l_tmb[:nb] = ltm[1::2]
        l2full = np.empty((P, npair * P), np.int16)
        l2full[:K] = np.broadcast_to(
            l_tma.reshape(1, npair * P), (K, npair * P)
        )
        l2full[K:] = np.broadcast_to(
            l_tmb.reshape(1, npair * P), (K, npair * P)
        )
        # iotarep [P, K, J1] int16: value k at [:, k, :]
        iotarep = np.tile(
            np.arange(K, dtype=np.int16)[None, :, None], (P, 1, J1)
        )
        # rank-2 one-hot operands for the Act-built pass-1 tail (last NACT
        # tiles): per tail tile r, rows [1; l] live at partitions
        # 2*(r%64), +1, column band r//64 (PAD remapped to 100; all values
        # exact in bf16). PE gives (k - l) exactly; Act Square + Relu(1-x)
        # recover the one-hot.
        n_act_t = min(NACT, tpc)
        lsm = np.where(lpj == PAD_LABEL, 100, lpj).T.astype(np.int64)  # [tpc, P]
        nbands = max(1, math.ceil(n_act_t / 3))
        lr3h = np.zeros((P, nbands * P), np.float32)
        for r in range(n_act_t):
            s, b = r % 3, r // 3  # slot partition offsets 0/32/64 only
            lr3h[32 * s, b * P : (b + 1) * P] = 1.0
            lr3h[32 * s + 1, b * P : (b + 1) * P] = lsm[r]
        lr3h = lr3h.astype(ml_dtypes.bfloat16)
        kv = np.arange(K, dtype=np.float32)
        kvecrep = np.zeros((P, K), np.float32)
        for s in range(3):
            kvecrep[32 * s] = kv
            kvecrep[32 * s + 1] = -1.0
        kvecrep = kvecrep.astype(ml_dtypes.bfloat16)  # [128, K]
        in_maps.append(
            {
                "xe": np.ascontiguousarray(xe),
                "labels_pm": np.ascontiguousarray(l_pm),
                "l2full": l2full,
                "iotarep": np.ascontiguousarray(iotarep),
                "lr3h": np.ascontiguousarray(lr3h),
                "kvecrep": np.ascontiguousarray(kvecrep),
                "iotacol2": np.concatenate(
                    [np.arange(K), np.arange(K)]
                ).astype(np.float32).reshape(P, 1),
                "id128": np.eye(P, dtype=ml_dtypes.bfloat16),
                "id33": np.eye(D + 1, dtype=np.float32),
                "id64": np.eye(K, dtype=np.float32),
                "eyeneg": (1.0 - np.eye(K, dtype=np.float32)).astype(
                    ml_dtypes.bfloat16
                ),
            }
        )
    return in_maps


def build_program(tpc, dve_sq_every=6, f1_dve_every=0, ht_bufs=7, l2_bufs=3, mg_bufs=4):
    """Build the SPMD Bass program. tpc = tiles per core."""
    nc = bacc.Bacc(
        "TRN2", target_bir_lowering=False, debug=False, num_devices=N_CORES
    )
    core_ids = list(range(N_CORES))
    npair = (tpc + 1) // 2

    xe_d = nc.dram_tensor("xe", [P, tpc, D + 1], BF16, kind="ExternalInput").ap()
    lpm_d = nc.dram_tensor("labels_pm", [P, tpc], I16, kind="ExternalInput").ap()
    l2f_d = nc.dram_tensor("l2full", [P, npair * P], I16, kind="ExternalInput").ap()
    iotarep_d = nc.dram_tensor("iotarep", [P, K, J1], I16, kind="ExternalInput").ap()
    n_act_tiles = min(NACT, tpc)
    n_dve_tiles = tpc - n_act_tiles
    nbands = max(1, math.ceil(n_act_tiles / 3))
    lr3h_d = nc.dram_tensor("lr3h", [P, nbands * P], BF16, kind="ExternalInput").ap()
    kvecrep_d = nc.dram_tensor("kvecrep", [P, K], BF16, kind="ExternalInput").ap()
    iotacol2_d = nc.dram_tensor("iotacol2", [P, 1], F32, kind="ExternalInput").ap()
    id128_d = nc.dram_tensor("id128", [P, P], BF16, kind="ExternalInput").ap()
    id33_d = nc.dram_tensor("id33", [D + 1, D + 1], F32, kind="ExternalInput").ap()
    id64_d = nc.dram_tensor("id64", [K, K], F32, kind="ExternalInput").ap()
    eyeneg_d = nc.dram_tensor("eyeneg", [K, K], BF16, kind="ExternalInput").ap()
    out_d = nc.dram_tensor("out", [3], F32, kind="ExternalOutput").ap()

    n_chunks1 = math.ceil(tpc / J1)
    n_oc = math.ceil(npair / TPAIR)

    with tile.TileContext(nc, num_cores=N_CORES) as tc, ExitStack() as ctx:
        singles = ctx.enter_context(tc.tile_pool(name="singles", bufs=1))
        xpool = ctx.enter_context(tc.tile_pool(name="xpool", bufs=1))
        h2pool = ctx.enter_context(tc.tile_pool(name="h2pool", bufs=3))
        hqpool = ctx.enter_context(tc.tile_pool(name="hqpool", bufs=2))
        l2pool = ctx.enter_context(tc.tile_pool(name="l2pool", bufs=l2_bufs))
        htpool = ctx.enter_context(tc.tile_pool(name="htpool", bufs=ht_bufs))
        sqpool = ctx.enter_context(tc.tile_pool(name="sqpool", bufs=3))  # sq/f1..f4/cpy tags
        wpool = ctx.enter_context(tc.tile_pool(name="wpool", bufs=2))
        psA = ctx.enter_context(tc.tile_pool(name="psA", bufs=1, space="PSUM"))
        psQp = ctx.enter_context(tc.tile_pool(name="psQp", bufs=1, space="PSUM"))
        psMg = ctx.enter_context(tc.tile_pool(name="psMg", bufs=mg_bufs, space="PSUM"))
        psS = ctx.enter_context(tc.tile_pool(name="psS", bufs=2, space="PSUM"))
        dram = ctx.enter_context(tc.tile_pool(name="dram", bufs=2, space="DRAM"))

        # ---------- constants ----------
        lpm = singles.tile([P, tpc], I16)
        nc.sync.dma_start(out=lpm, in_=lpm_d)
        iotarep = singles.tile([P, K, J1], I16)
        nc.sync.dma_start(out=iotarep, in_=iotarep_d)
        iotacol2 = singles.tile([P, 1], F32)
        nc.sync.dma_start(out=iotacol2, in_=iotacol2_d)
        id128 = singles.tile([P, P], BF16)
        nc.sync.dma_start(out=id128, in_=id128_d)
        id33 = singles.tile([D + 1, D + 1], F32)
        nc.sync.dma_start(out=id33, in_=id33_d)
        id64 = singles.tile([K, K], F32)
        nc.sync.dma_start(out=id64, in_=id64_d)
        eyeneg = singles.tile([K, K], BF16)
        nc.sync.dma_start(out=eyeneg, in_=eyeneg_d)
        d2all = singles.tile([P, tpc], F32)
        invc_all = singles.tile([P, tpc], BF16)
        hh = singles.tile([P, tpc], F32)
        hhw = singles.tile([P, tpc], F32)
        rsacc = singles.tile([P, n_oc], F32)
        margneg = singles.tile([P, 1], F32)
        nc.gpsimd.memset(margneg, -float(INTRA_MARGIN))

        # l2 chunk DMA helper (host-materialized paired label broadcast)
        def issue_l2(oc):
            t0 = oc * TPAIR
            tn = min(TPAIR, npair - t0)
            l2 = l2pool.tile([P, TPAIR * P], I16, tag="l2")
            nc.sync.dma_start(
                out=l2[:, : tn * P], in_=l2f_d[:, t0 * P : (t0 + tn) * P]
            )
            return l2, tn

        # ---------- pass 1: segment sums ----------
        # xe DMAs issued first so pass-1 is never starved by the (large)
        # l2 broadcast transfers; l2 chunks are issued after so the
        # collective is not queued behind them on the DMA engines.
        # The LAST n_act_tiles tiles use an Act-engine one-hot instead of
        # DVE: PE rank-3 matmul gives (k - l)^2 in PSUM, Act relu(1 - x)
        # turns it into the one-hot (Act is otherwise idle before the
        # collective; this shortens the DVE-bound pass-1 phase).
        lr3 = singles.tile([P, nbands * P], BF16)
        nc.sync.dma_start(out=lr3, in_=lr3h_d)
        kvec = singles.tile([P, K], BF16)
        nc.sync.dma_start(out=kvec, in_=kvecrep_d)
        psumS = psA.tile([D + 1, K], F32)
        l2_tiles = []
        lc = 0
        t_done = 0
        # Act-built one-hot groups (7 tiles per PSUM bank), interleaved
        # among the DVE-built chunks so the PE queue never stalls long on
        # the PE->Act->PE round trip; Act is otherwise idle pre-collective.
        JQ = 7
        n_qgroups = math.ceil(n_act_tiles / JQ)

        def emit_act_group(qg):
            global_t = globals()  # noqa - placeholder
        def act_group(qg, t_done):
            q0 = qg * JQ
            qn = min(JQ, n_act_tiles - q0)
            nc.sync.dma_start(
                out=xe[:, q0 : q0 + qn, :], in_=xe_d[:, q0 : q0 + qn, :]
            )
            psQ = psQp.tile([P, JQ, K], F32, tag="psq")
            for t in range(qn):
                r = q0 + t
                s, b = r % 3, r // 3
                nc.tensor.matmul(
                    psQ[:, t, :],
                    lr3[32 * s : 32 * s + 2, b * P : (b + 1) * P],
                    kvec[32 * s : 32 * s + 2, :],
                    start=True, stop=True,
                )
            h2sq = hqpool.tile([P, JQ, K], BF16, tag="h2sq")
            nc.scalar.activation(
                out=h2sq[:, :qn, :], in_=psQ[:, :qn, :],
                func=mybir.ActivationFunctionType.Square,
            )
            h2a = hqpool.tile([P, JQ, K], BF16, tag="h2a")
            nc.scalar.activation(
                out=h2a[:, :qn, :], in_=h2sq[:, :qn, :],
                func=mybir.ActivationFunctionType.Relu, bias=1.0, scale=-1.0,
            )
            for t in range(qn):
                nc.tensor.matmul(
                    psumS,
                    xe[:, q0 + t, :],
                    h2a[:, t, :],
                    start=(t_done == 0),
                    stop=(t_done == tpc - 1),
                )
                t_done += 1
            return t_done

        # DVE-built chunks; xe is streamed through a rolling pool (the
        # full-size resident copy is gone — pass 2 re-streams its own xe
        # chunks during the otherwise idle collective window, freeing
        # ~64KB of SBUF for a much deeper ht pool)
        n_chunks1d = math.ceil(n_dve_tiles / J1)
        qg_next = 0
        xe = xpool.tile([P, tpc, D + 1], BF16)
        for c in range(n_chunks1d):
            j0 = n_act_tiles + c * J1
            jn = min(J1, tpc - j0)
            nc.sync.dma_start(
                out=xe[:, j0 : j0 + jn, :], in_=xe_d[:, j0 : j0 + jn, :]
            )
            h2 = h2pool.tile([P, K, J1], BF16, tag="h2")
            nc.vector.tensor_tensor(
                h2[:, :, :jn],
                lpm[:, None, j0 : j0 + jn].to_broadcast((P, K, jn)),
                iotarep[:, :, :jn],
                mybir.AluOpType.is_equal,
            )
            for j in range(jn):
                nc.tensor.matmul(
                    psumS,
                    xe[:, j0 + j, :],
                    h2[:, :, j],
                    start=(t_done == 0),
                    stop=(t_done == tpc - 1),
                )
                t_done += 1
        # l2 label chunks stream right behind xe on the DMA engines
        while lc < n_oc:
            l2_tiles.append(issue_l2(lc))
            lc += 1
        # ---------- AllGather segment sums + local reduce ----------
        # (AllGather avoids the cost model's 1.875x AllReduce penalty; the
        #  8-way sum is 3 cheap tree adds done locally)
        sg_local = wpool.tile([D + 1, K], F32, tag="sg")
        nc.scalar.copy(out=sg_local, in_=psumS)
        cc_in = dram.tile([D + 1, K], F32)
        cc_out = dram.tile([N_CORES, D + 1, K], F32)
        nc.gpsimd.dma_start(out=cc_in, in_=sg_local)
        nc.gpsimd.collective_compute(
            "AllGather",
            mybir.AluOpType.bypass,
            replica_groups=[core_ids],
            ins=[cc_in.opt()],
            outs=[cc_out.opt()],
        )
        sg8 = wpool.tile([D + 1, N_CORES, K], F32, tag="sg8")
        ccf = cc_out[0, 0, 0]  # base AP for offset/tensor
        nc.gpsimd.dma_start(
            out=sg8,
            in_=bass.AP(
                tensor=ccf.tensor, offset=ccf.offset,
                ap=[[K, D + 1], [(D + 1) * K, N_CORES], [1, K]],
            ),
        )
        sg4 = wpool.tile([D + 1, 4, K], F32, tag="sg4")
        nc.vector.tensor_add(sg4, sg8[:, :4, :], sg8[:, 4:, :])
        sg2t = wpool.tile([D + 1, 2, K], F32, tag="sg2t")
        nc.vector.tensor_add(sg2t, sg4[:, :2, :], sg4[:, 2:, :])
        sg = wpool.tile([D + 1, K], F32, tag="sg2")
        nc.vector.tensor_tensor(
            sg, sg2t[:, 0, :], sg2t[:, 1, :], mybir.AluOpType.add
        )

        # ---------- ht builds (no AR dependency) ----------
        ht_tiles = []
        for oc in range(n_oc):
            l2, tn = l2_tiles[oc]
            ht = htpool.tile([P, TPAIR * P], BF16, tag="ht")
            nc.vector.tensor_single_scalar(
                ht[:, : tn * P], l2[:, : tn * P], iotacol2,
                mybir.AluOpType.is_equal,
            )
            ht_tiles.append(ht)

        # ---------- stats (Act/Pool/PE only; DVE stays on one-hot work) ----
        psW = psS.tile([K, D + 1], F32, tag="small")
        nc.tensor.transpose(psW, sg, id33)
        W = wpool.tile([K, D + 1], F32, tag="w")  # [S_k | c_k]
        nc.scalar.copy(out=W, in_=psW)
        safec = wpool.tile([K, 1], F32, tag="safec")
        nc.gpsimd.tensor_scalar(
            safec, W[:, D : D + 1], 256.0, 1.0,
            mybir.AluOpType.mult, mybir.AluOpType.max,
        )
        invc = wpool.tile([K, 1], F32, tag="invc")
        nc.vector.reciprocal(invc, safec)
        svp = wpool.tile([K, 1], F32, tag="svp")  # sqrt(1/c)
        nc.scalar.activation(
            out=svp, in_=invc, func=mybir.ActivationFunctionType.Sqrt
        )
        mu = wpool.tile([K, D], F32, tag="mu")
        nc.gpsimd.tensor_mul(mu, W[:, :D], invc.to_broadcast((K, D)))
        # table2 [128, 33] bf16 = [eps - mu | sqrt(1/c) - 1], rows replicated
        table2 = singles.tile([P, D + 1], BF16)
        nc.scalar.activation(
            out=table2[:K, :D], in_=mu,
            func=mybir.ActivationFunctionType.Copy, bias=EPS, scale=-1.0,
        )
        nc.scalar.activation(
            out=table2[:K, D : D + 1], in_=svp,
            func=mybir.ActivationFunctionType.Copy, bias=-1.0 / 256.0,
        )
        nc.sync.dma_start(out=table2[K:, :], in_=table2[:K, :])

        # ---------- inter + reg losses (Act/Pool/PE) ----------
        mup = wpool.tile([K, D], F32, tag="mup")  # mu + eps
        nc.scalar.activation(
            out=mup, in_=mu, func=mybir.ActivationFunctionType.Copy, bias=EPS
        )
        qsc = wpool.tile([K, D], F32, tag="qsc")
        nc.gpsimd.tensor_mul(qsc, mu, mu)
        q = wpool.tile([K, 1], F32, tag="q")  # ||mu||^2
        nc.vector.tensor_reduce(
            out=q, in_=qsc, axis=mybir.AxisListType.X, op=mybir.AluOpType.add
        )
        qpsc = wpool.tile([K, D], F32, tag="qpsc")
        nc.gpsimd.tensor_mul(qpsc, mup, mup)
        qp = wpool.tile([K, 1], F32, tag="qp")  # ||mu + eps||^2
        nc.vector.tensor_reduce(
            out=qp, in_=qpsc, axis=mybir.AxisListType.X, op=mybir.AluOpType.add
        )
        # pd2[a,b] = qp_a - 2*mup_a.mu_b + q_b via [ -2*mup | qp | 1 ] x [ mu | 1 | q ]
        ab = wpool.tile([K, D + 2], F32, tag="ab")
        nc.scalar.mul(out=ab[:, :D], in_=mup, mul=-2.0)
        nc.scalar.copy(out=ab[:, D : D + 1], in_=qp)
        nc.gpsimd.memset(ab[:, D + 1 : D + 2], 1.0)
        bb = wpool.tile([K, D + 2], F32, tag="bb")
        nc.scalar.copy(out=bb[:, :D], in_=mu)
        nc.gpsimd.memset(bb[:, D : D + 1], 1.0)
        nc.scalar.copy(out=bb[:, D + 1 : D + 2], in_=q)
        psT = psS.tile([D + 2, K], F32, tag="small")
        nc.tensor.transpose(psT, ab, id64)
        atp = wpool.tile([D + 2, K], F32, tag="atp")
        nc.scalar.copy(out=atp, in_=psT)
        psT2 = psS.tile([D + 2, K], F32, tag="small")
        nc.tensor.transpose(psT2, bb, id64)
        btp = wpool.tile([D + 2, K], F32, tag="btp")
        nc.scalar.copy(out=btp, in_=psT2)
        psPD = psS.tile([K, K], F32, tag="small")
        nc.tensor.matmul(psPD, atp, btp)
        pdc = wpool.tile([K, K], F32, tag="pdc")
        nc.vector.tensor_scalar_max(pdc, psPD, 0.0)
        pdist = wpool.tile([K, K], F32, tag="pdist")
        nc.scalar.activation(
            out=pdist, in_=pdc, func=mybir.ActivationFunctionType.Sqrt
        )
        hingeI = wpool.tile([K, K], F32, tag="hingeI")
        nc.scalar.activation(
            out=hingeI, in_=pdist, func=mybir.ActivationFunctionType.Relu,
            bias=float(INTER_MARGIN2), scale=-1.0,
        )
        hm = wpool.tile([K, K], F32, tag="hm")
        nc.gpsimd.tensor_mul(hm, hingeI, eyeneg)
        hm2 = wpool.tile([K, K], F32, tag="hm2")
        nc.gpsimd.tensor_mul(hm2, hm, hm)
        interp = wpool.tile([K, 1], F32, tag="interp")
        nc.vector.tensor_reduce(
            out=interp, in_=hm2, axis=mybir.AxisListType.X,
            op=mybir.AluOpType.add,
        )
        sqp = wpool.tile([K, 1], F32, tag="sqp")  # ||mu + eps||
        nc.scalar.activation(
            out=sqp, in_=qp, func=mybir.ActivationFunctionType.Sqrt
        )
        cat2 = wpool.tile([K, 2], F32, tag="cat2")
        nc.scalar.copy(out=cat2[:, 0:1], in_=interp)
        nc.scalar.copy(out=cat2[:, 1:2], in_=sqp)
        ones64 = singles.tile([K, 1], F32)
        nc.gpsimd.memset(ones64, 1.0)
        psIR = psS.tile([1, 2], F32, tag="small")
        nc.tensor.matmul(psIR, ones64, cat2)
        ir = wpool.tile([1, 2], F32, tag="ir")  # [inter_sum, reg_sum]
        nc.scalar.copy(out=ir, in_=psIR)

        # ---------- pass 2: gather + diff in PSUM, square, fold-reduce ------
        fin_oc = sorted(set(
            [n_oc - 1] + [max(0, (n_oc * (q + 1)) // 4 - 1) for q in range(3)]
        ))
        fin_base = []
        prev = 0
        for oc_ in fin_oc:
            fin_base.append(prev)
            pc_ = min(TPAIR, npair - oc_ * TPAIR)
            prev = min(oc_ * TPAIR * 2 + pc_ * 2, tpc)
        for oc in range(n_oc):
            ht = ht_tiles[oc]
            pc = min(TPAIR, npair - oc * TPAIR)
            cbase = oc * TPAIR * 2        # first global tile of this chunk
            ctn = min(pc * 2, tpc - cbase)  # tiles in this chunk
            # one sq tile per ht chunk (up to 56 tiles), 4 PSUM groups
            sq = sqpool.tile([P, TPAIR * 2, D + 1], BF16, tag="sq")
            for g in range(math.ceil(pc / (JMG // 2))):
                p0 = g * (JMG // 2)
                pn = min(JMG // 2, pc - p0)
                jbase = (oc * TPAIR + p0) * 2  # first global tile of group
                nt = min(pn * 2, tpc - jbase)
                psD = psMg.tile([P, JMG, D + 1], F32, tag="psd")
                for lp in range(pn):
                    for half in range(2):
                        t = lp * 2 + half
                        if t >= nt:
                            break
                        colp = p0 + lp
                        nc.tensor.matmul(
                            psD[:, t, :],
                            ht[64 * half : 64 * (half + 1),
                               colp * P : (colp + 1) * P],
                            table2[64 * half : 64 * (half + 1), :],
                            start=True, stop=False,
                        )
                        nc.tensor.matmul(
                            psD[:, t, :], id128,
                            xe[:, cbase + p0 * 2 + t, :],
                            start=False, stop=True,
                        )
                t0 = p0 * 2
                if dve_sq_every and (oc * 4 + g) % dve_sq_every == dve_sq_every - 1:
                    # DVE square: PSUM copy then bf16 self-mult (one PSUM
                    # input per instruction as required by hardware)
                    cpy = sqpool.tile([P, JMG, D + 1], BF16, tag="cpy")
                    nc.vector.tensor_scalar_add(
                        cpy[:, :nt, :], psD[:, :nt, :], 0.0
                    )
                    nc.vector.tensor_mul(
                        sq[:, t0 : t0 + nt, :], cpy[:, :nt, :], cpy[:, :nt, :]
                    )
                else:
                    nc.scalar.activation(
                        out=sq[:, t0 : t0 + nt, :], in_=psD[:, :nt, :],
                        func=mybir.ActivationFunctionType.Square,
                    )
            # bf16 fold-tree reduce over D (2x DVE mode); f1 of every other
            # chunk goes to Pool to offload DVE
            with nc.allow_low_precision(reason="bf16 partial sums of d2"):
                f1 = sqpool.tile([P, TPAIR * 2, 16], BF16, tag="f1")
                f1eng = nc.vector if (f1_dve_every and oc % f1_dve_every == f1_dve_every - 1) else nc.gpsimd
                f1eng.tensor_add(
                    f1[:, :ctn, :], sq[:, :ctn, 0:16], sq[:, :ctn, 16:32]
                )
                f2 = sqpool.tile([P, TPAIR * 2, 8], BF16, tag="f2")
                nc.vector.tensor_add(
                    f2[:, :ctn, :], f1[:, :ctn, 0:8], f1[:, :ctn, 8:16]
                )
                f3 = sqpool.tile([P, TPAIR * 2, 4], BF16, tag="f3")
                nc.vector.tensor_add(
                    f3[:, :ctn, :], f2[:, :ctn, 0:4], f2[:, :ctn, 4:8]
                )
                f4 = sqpool.tile([P, TPAIR * 2, 2], BF16, tag="f4")
                nc.vector.tensor_add(
                    f4[:, :ctn, :], f3[:, :ctn, 0:2], f3[:, :ctn, 2:4]
                )
            nc.vector.tensor_tensor(
                d2all[:, cbase : cbase + ctn],
                f4[:, :ctn, 0], f4[:, :ctn, 1], mybir.AluOpType.add,
            )
            nc.gpsimd.tensor_scalar_add(
                invc_all[:, cbase : cbase + ctn], sq[:, :ctn, D], 0.0
            )
            # quarter-granularity finals (keeps the serial tail short
            # without flooding Act with per-chunk overhead):
            # dist = sqrt(d2); h = relu(dist - 1.5); acc_q = sum h^2 * invc
            if oc in fin_oc:
                qi = fin_oc.index(oc)
                b0 = fin_base[qi]
                b1 = cbase + ctn
                dsl = d2all[:, b0:b1]
                nc.scalar.activation(
                    out=dsl, in_=dsl,
                    func=mybir.ActivationFunctionType.Sqrt,
                )
                nc.scalar.activation(
                    out=dsl, in_=dsl,
                    func=mybir.ActivationFunctionType.Relu, bias=margneg,
                )
                hsl = hh[:, b0:b1]
                nc.vector.tensor_mul(hsl, dsl, dsl)
                wsl = hhw[:, b0:b1]
                nc.vector.tensor_mul(wsl, hsl, invc_all[:, b0:b1])
                nc.vector.tensor_reduce(
                    out=rsacc[:, qi : qi + 1], in_=wsl,
                    axis=mybir.AxisListType.X, op=mybir.AluOpType.add,
                )

        # ---------- finals: reduce per-quarter partials ----------
        rowsum = singles.tile([P, 1], F32)
        nc.vector.tensor_reduce(
            out=rowsum, in_=rsacc[:, : len(fin_oc)],
            axis=mybir.AxisListType.X, op=mybir.AluOpType.add,
        )
        ones128 = singles.tile([P, 1], F32)
        nc.gpsimd.memset(ones128, 1.0)
        psL = psS.tile([1, 1], F32, tag="small")
        nc.tensor.matmul(psL, rowsum, ones128)
        tot = wpool.tile([1, 3], F32, tag="tot")
        nc.scalar.copy(out=tot[:, 0:1], in_=psL)
        nc.scalar.copy(out=tot[:, 1:3], in_=ir)
        nc.sync.dma_start(out=out_d, in_=tot[0:1, :])

    nc.compile()
    return nc


_NC_CACHE = {}


def _get_program(tpc):
    if tpc not in _NC_CACHE:
        _NC_CACHE[tpc] = build_program(tpc)
    return _NC_CACHE[tpc]


def kernel(features, labels, num_clusters):
    features = np.asarray(features)
    labels = np.asarray(labels)
    n_total = features.shape[0]
    n_core = n_total // N_CORES
    tpc = math.ceil(n_core / P)
    nc = _get_program(tpc)
    in_maps = _host_prep(features, labels, tpc)
    res = run_bass_kernel_spmd(nc, in_maps, list(range(N_CORES)))
    intra_sum = sum(float(res.results[c]["out"][0]) for c in range(N_CORES))
    inter_sum = float(res.results[0]["out"][1])
    reg_sum = float(res.results[0]["out"][2])
    total = (
        intra_sum / K
        + inter_sum / (K * (K - 1))
        + 0.001 * reg_sum / K
    )
    return np.float32(total)


# revision 97
# speedup vs baseline: 1.0023x; 1.0023x over previous
# kernel.py — DiscriminativeLoss on 8 TRN2 NeuronCores (Bass/Tile, SPMD).
#
# Math (matches reference):
#   counts_k = #{i: l_i = k};  S_k = sum_{i in k} x_i;  mu_k = S_k / max(c_k, 1)
#   intra = (1/K) * sum_i invc_{l_i} * relu(||x_i - mu_{l_i} + eps|| - 1.5)^2
#   inter = sum_{a != b} relu(1 - ||(mu_a + eps) - mu_b||)^2 / (K*(K-1))
#   reg   = (1/K) * sum_k ||mu_k + eps||
#   total = intra + inter + 0.001 * reg
#
# V2 design (engine-balanced, cost-model driven):
#   pass 1: one-hot H2 [P, K, jn] via DVE/Pool tensor_tensor is_equal (2x
#     mode: all operands 2-byte packed SBUF); PE matmul lhsT=Xe [128,33],
#     rhs=H2[:, :, j] accumulates S^T = [S | counts] in PSUM [33, 64].
#   AllReduce [33, 64]; stats (mu, 1/c, sqrt(1/c), inter/reg losses) on
#     Act/Pool/PE only, keeping DVE free.
#   pass 2: paired transposed one-hot ht [128, 128] per tile-pair (A on
#     partitions 0-63, B on 64-127) built from broadcast-DMA'd labels via
#     DVE tensor_single_scalar is_equal (4x mode). Per tile, TWO accumulating
#     matmuls produce diff = x - (mu - eps) directly in PSUM:
#       psD  = ht_half^T @ [eps - mu | sqrt(1/c) - 1]   (gather, negated)
#       psD += I_128    @ Xe_tile                        ([x | 1])
#     -> psD = [x - mu + eps | sqrt(1/c)].
#   Act Square psD -> sq bf16 (col 32 squares to 1/c); DVE/Pool tensor_reduce
#     over D -> d2; DVE copies col 32 -> invc_all.
#   finals: dist=sqrt(d2), h=relu(dist-1.5) on Act; intra partial
#     sum_i h^2 * invc via DVE mults + tensor_tensor_reduce + PE ones-matmul.
import math
import numpy as np
from contextlib import ExitStack

import concourse.bass as bass
import concourse.bacc as bacc
import concourse.tile as tile
import concourse.mybir as mybir
from concourse.bass_utils import run_bass_kernel_spmd

F32 = mybir.dt.float32
BF16 = mybir.dt.bfloat16
I16 = mybir.dt.int16

N_CORES = 8
K = 64
D = 32
P = 128
EPS = 1e-8
PAD_LABEL = 999  # never matches any one-hot column

INTRA_MARGIN = 1.5
INTER_MARGIN2 = 1.0  # 2 * 0.5

J1 = 20      # pass-1 chunk width (tiles)
NACT = 0     # pass-1 tiles whose one-hot is built on Act (PE rank-3 + relu)
TPAIR = 28   # tile-pairs per ht chunk (4 gather groups of 7 pairs)
JMG = 14     # tiles per PSUM gather group (7 pairs)


def _host_prep(features, labels, tpc):
    """Shard + relayout on host. Returns per-core input dicts."""
    n_total = features.shape[0]
    n_core = n_total // N_CORES
    n_pad = P * tpc
    npair = (tpc + 1) // 2
    import ml_dtypes

    in_maps = []
    for c in range(N_CORES):
        f = np.asarray(features[c * n_core : (c + 1) * n_core], dtype=np.float32)
        l = np.asarray(labels[c * n_core : (c + 1) * n_core], dtype=np.int64)
        if n_pad > n_core:
            f = np.concatenate([f, np.zeros((n_pad - n_core, D), np.float32)], axis=0)
            l = np.concatenate([l, np.full((n_pad - n_core,), PAD_LABEL, np.int64)])
        # Xe: [P, tpc, 33] bf16, col 32 = 1/256 (exact in bf16; keeps the
        # sqrt(1/c) gather free of bf16 cancellation); point i = (i%P, i//P)
        xe = np.full((n_pad, D + 1), 1.0 / 256.0, np.float32)
        xe[:, :D] = f
        xe = xe.reshape(P, tpc, D + 1).astype(ml_dtypes.bfloat16)
        lpj = l.reshape(P, tpc)  # [point-in-tile, tile]
        l_pm = lpj.astype(np.int16)
        # paired label broadcast for ht, fully materialized on host:
        # [128, npair*128] int16, rows 0-63 = labels of tile 2jj, rows
        # 64-127 = labels of tile 2jj+1 (one contiguous DMA per chunk)
        ltm = lpj.T.astype(np.int16)  # [tpc, P]
        l_tma = np.full((npair, P), PAD_LABEL, np.int16)
        l_tmb = np.full((npair, P), PAD_LABEL, np.int16)
        l_tma[:] = ltm[0::2]
        nb = tpc // 2
        l_tmb[:nb] = ltm[1::2]
        l2full = np.empty((P, npair * P), np.int16)
        l2full[:K] = np.broadcast_to(
            l_tma.reshape(1, npair * P), (K, npair * P)
        )
        l2full[K:] = np.broadcast_to(
            l_tmb.reshape(1, npair * P), (K, npair * P)
        )
        # iotarep [P, K, J1] int16: value k at [:, k, :]
        iotarep = np.tile(
            np.arange(K, dtype=np.int16)[None, :, None], (P, 1, J1)
        )
        # rank-2 one-hot operands for the Act-built pass-1 tail (last NACT
        # tiles): per tail tile r, rows [1; l] live at partitions
        # 2*(r%64), +1, column band r//64 (PAD remapped to 100; all values
        # exact in bf16). PE gives (k - l) exactly; Act Square + Relu(1-x)
        # recover the one-hot.
        n_act_t = min(NACT, tpc)
        lsm = np.where(lpj == PAD_LABEL, 100, lpj).T.astype(np.int64)  # [tpc, P]
        nbands = max(1, math.ceil(n_act_t / 3))
        lr3h = np.zeros((P, nbands * P), np.float32)
        for r in range(n_act_t):
            s, b = r % 3, r // 3  # slot partition offsets 0/32/64 only
            lr3h[32 * s, b * P : (b + 1) * P] = 1.0
            lr3h[32 * s + 1, b * P : (b + 1) * P] = lsm[r]
        lr3h = lr3h.astype(ml_dtypes.bfloat16)
        kv = np.arange(K, dtype=np.float32)
        kvecrep = np.zeros((P, K), np.float32)
        for s in range(3):
            kvecrep[32 * s] = kv
            kvecrep[32 * s + 1] = -1.0
        kvecrep = kvecrep.astype(ml_dtypes.bfloat16)  # [128, K]
        in_maps.append(
            {
                "xe": np.ascontiguousarray(xe),
                "labels_pm": np.ascontiguousarray(l_pm),
                "l2full": l2full,
                "iotarep": np.ascontiguousarray(iotarep),
                "lr3h": np.ascontiguousarray(lr3h),
                "kvecrep": np.ascontiguousarray(kvecrep),
                "iotacol2": np.concatenate(
                    [np.arange(K), np.arange(K)]
                ).astype(np.float32).reshape(P, 1),
                "id128": np.eye(P, dtype=ml_dtypes.bfloat16),
                "idrep": np.ascontiguousarray(np.hstack(
                    [np.eye(K), np.eye(K)]).astype(ml_dtypes.bfloat16)),
                "id33": np.eye(D + 1, dtype=np.float32),
                "id64": np.eye(K, dtype=np.float32),
                "eyeneg": (1.0 - np.eye(K, dtype=np.float32)).astype(
                    ml_dtypes.bfloat16
                ),
            }
        )
    return in_maps


def build_program(tpc, dve_sq_every=6, f1_dve_every=0, ht_bufs=7, l2_bufs=3, mg_bufs=4):
    """Build the SPMD Bass program. tpc = tiles per core."""
    nc = bacc.Bacc(
        "TRN2", target_bir_lowering=False, debug=False, num_devices=N_CORES
    )
    core_ids = list(range(N_CORES))
    npair = (tpc + 1) // 2

    xe_d = nc.dram_tensor("xe", [P, tpc, D + 1], BF16, kind="ExternalInput").ap()
    lpm_d = nc.dram_tensor("labels_pm", [P, tpc], I16, kind="ExternalInput").ap()
    l2f_d = nc.dram_tensor("l2full", [P, npair * P], I16, kind="ExternalInput").ap()
    iotarep_d = nc.dram_tensor("iotarep", [P, K, J1], I16, kind="ExternalInput").ap()
    n_act_tiles = min(NACT, tpc)
    n_dve_tiles = tpc - n_act_tiles
    nbands = max(1, math.ceil(n_act_tiles / 3))
    lr3h_d = nc.dram_tensor("lr3h", [P, nbands * P], BF16, kind="ExternalInput").ap()
    kvecrep_d = nc.dram_tensor("kvecrep", [P, K], BF16, kind="ExternalInput").ap()
    iotacol2_d = nc.dram_tensor("iotacol2", [P, 1], F32, kind="ExternalInput").ap()
    id128_d = nc.dram_tensor("id128", [P, P], BF16, kind="ExternalInput").ap()
    idrep_d = nc.dram_tensor("idrep", [K, P], BF16, kind="ExternalInput").ap()
    id33_d = nc.dram_tensor("id33", [D + 1, D + 1], F32, kind="ExternalInput").ap()
    id64_d = nc.dram_tensor("id64", [K, K], F32, kind="ExternalInput").ap()
    eyeneg_d = nc.dram_tensor("eyeneg", [K, K], BF16, kind="ExternalInput").ap()
    out_d = nc.dram_tensor("out", [3], F32, kind="ExternalOutput").ap()

    n_chunks1 = math.ceil(tpc / J1)
    n_oc = math.ceil(npair / TPAIR)

    with tile.TileContext(nc, num_cores=N_CORES) as tc, ExitStack() as ctx:
        singles = ctx.enter_context(tc.tile_pool(name="singles", bufs=1))
        xpool = ctx.enter_context(tc.tile_pool(name="xpool", bufs=1))
        h2pool = ctx.enter_context(tc.tile_pool(name="h2pool", bufs=3))
        hqpool = ctx.enter_context(tc.tile_pool(name="hqpool", bufs=2))
        l2pool = ctx.enter_context(tc.tile_pool(name="l2pool", bufs=l2_bufs))
        htpool = ctx.enter_context(tc.tile_pool(name="htpool", bufs=ht_bufs))
        sqpool = ctx.enter_context(tc.tile_pool(name="sqpool", bufs=3))  # sq/f1..f4/cpy tags
        wpool = ctx.enter_context(tc.tile_pool(name="wpool", bufs=2))
        psA = ctx.enter_context(tc.tile_pool(name="psA", bufs=1, space="PSUM"))
        psQp = ctx.enter_context(tc.tile_pool(name="psQp", bufs=1, space="PSUM"))
        psMg = ctx.enter_context(tc.tile_pool(name="psMg", bufs=mg_bufs, space="PSUM"))
        psS = ctx.enter_context(tc.tile_pool(name="psS", bufs=2, space="PSUM"))
        dram = ctx.enter_context(tc.tile_pool(name="dram", bufs=2, space="DRAM"))

        # ---------- constants ----------
        lpm = singles.tile([P, tpc], I16)
        nc.sync.dma_start(out=lpm, in_=lpm_d)
        iotarep = singles.tile([P, K, J1], I16)
        nc.sync.dma_start(out=iotarep, in_=iotarep_d)
        iotacol2 = singles.tile([P, 1], F32)
        nc.sync.dma_start(out=iotacol2, in_=iotacol2_d)
        id128 = singles.tile([P, P], BF16)
        nc.sync.dma_start(out=id128, in_=id128_d)
        idrep = singles.tile([K, P], BF16)
        nc.sync.dma_start(out=idrep, in_=idrep_d)
        id33 = singles.tile([D + 1, D + 1], F32)
        nc.sync.dma_start(out=id33, in_=id33_d)
        id64 = singles.tile([K, K], F32)
        nc.sync.dma_start(out=id64, in_=id64_d)
        eyeneg = singles.tile([K, K], BF16)
        nc.sync.dma_start(out=eyeneg, in_=eyeneg_d)
        d2all = singles.tile([P, tpc], F32)
        invc_all = singles.tile([P, tpc], BF16)
        hh = singles.tile([P, tpc], F32)
        hhw = singles.tile([P, tpc], F32)
        rsacc = singles.tile([P, n_oc], F32)
        margneg = singles.tile([P, 1], F32)
        nc.gpsimd.memset(margneg, -float(INTRA_MARGIN))

        # l2 chunk DMA helper (host-materialized paired label broadcast)
        def issue_l2(oc):
            t0 = oc * TPAIR
            tn = min(TPAIR, npair - t0)
            l2 = l2pool.tile([P, TPAIR * P], I16, tag="l2")
            nc.sync.dma_start(
                out=l2[:, : tn * P], in_=l2f_d[:, t0 * P : (t0 + tn) * P]
            )
            return l2, tn

        # ---------- pass 1: segment sums ----------
        # xe DMAs issued first so pass-1 is never starved by the (large)
        # l2 broadcast transfers; l2 chunks are issued after so the
        # collective is not queued behind them on the DMA engines.
        # The LAST n_act_tiles tiles use an Act-engine one-hot instead of
        # DVE: PE rank-3 matmul gives (k - l)^2 in PSUM, Act relu(1 - x)
        # turns it into the one-hot (Act is otherwise idle before the
        # collective; this shortens the DVE-bound pass-1 phase).
        lr3 = singles.tile([P, nbands * P], BF16)
        nc.sync.dma_start(out=lr3, in_=lr3h_d)
        kvec = singles.tile([P, K], BF16)
        nc.sync.dma_start(out=kvec, in_=kvecrep_d)
        psumS = psA.tile([D + 1, K], F32)
        l2_tiles = []
        lc = 0
        t_done = 0
        # Act-built one-hot groups (7 tiles per PSUM bank), interleaved
        # among the DVE-built chunks so the PE queue never stalls long on
        # the PE->Act->PE round trip; Act is otherwise idle pre-collective.
        JQ = 7
        n_qgroups = math.ceil(n_act_tiles / JQ)

        def emit_act_group(qg):
            global_t = globals()  # noqa - placeholder
        def act_group(qg, t_done):
            q0 = qg * JQ
            qn = min(JQ, n_act_tiles - q0)
            nc.sync.dma_start(
                out=xe[:, q0 : q0 + qn, :], in_=xe_d[:, q0 : q0 + qn, :]
            )
            psQ = psQp.tile([P, JQ, K], F32, tag="psq")
            for t in range(qn):
                r = q0 + t
                s, b = r % 3, r // 3
                nc.tensor.matmul(
                    psQ[:, t, :],
                    lr3[32 * s : 32 * s + 2, b * P : (b + 1) * P],
                    kvec[32 * s : 32 * s + 2, :],
                    start=True, stop=True,
                )
            h2sq = hqpool.tile([P, JQ, K], BF16, tag="h2sq")
            nc.scalar.activation(
                out=h2sq[:, :qn, :], in_=psQ[:, :qn, :],
                func=mybir.ActivationFunctionType.Square,
            )
            h2a = hqpool.tile([P, JQ, K], BF16, tag="h2a")
            nc.scalar.activation(
                out=h2a[:, :qn, :], in_=h2sq[:, :qn, :],
                func=mybir.ActivationFunctionType.Relu, bias=1.0, scale=-1.0,
            )
            for t in range(qn):
                nc.tensor.matmul(
                    psumS,
                    xe[:, q0 + t, :],
                    h2a[:, t, :],
                    start=(t_done == 0),
                    stop=(t_done == tpc - 1),
                )
                t_done += 1
            return t_done

        # DVE-built chunks; xe is streamed through a rolling pool (the
        # full-size resident copy is gone — pass 2 re-streams its own xe
        # chunks during the otherwise idle collective window, freeing
        # ~64KB of SBUF for a much deeper ht pool)
        n_chunks1d = math.ceil(n_dve_tiles / J1)
        qg_next = 0
        xe = xpool.tile([P, tpc, D + 1], BF16)
        for c in range(n_chunks1d):
            j0 = n_act_tiles + c * J1
            jn = min(J1, tpc - j0)
            nc.sync.dma_start(
                out=xe[:, j0 : j0 + jn, :], in_=xe_d[:, j0 : j0 + jn, :]
            )
            h2 = h2pool.tile([P, K, J1], BF16, tag="h2")
            nc.vector.tensor_tensor(
                h2[:, :, :jn],
                lpm[:, None, j0 : j0 + jn].to_broadcast((P, K, jn)),
                iotarep[:, :, :jn],
                mybir.AluOpType.is_equal,
            )
            for j in range(jn):
                nc.tensor.matmul(
                    psumS,
                    xe[:, j0 + j, :],
                    h2[:, :, j],
                    start=(t_done == 0),
                    stop=(t_done == tpc - 1),
                )
                t_done += 1
        # l2 label chunks stream right behind xe on the DMA engines
        while lc < n_oc:
            l2_tiles.append(issue_l2(lc))
            lc += 1
        # ---------- AllGather segment sums + local reduce ----------
        # (AllGather avoids the cost model's 1.875x AllReduce penalty; the
        #  8-way sum is 3 cheap tree adds done locally)
        sg_local = wpool.tile([D + 1, K], F32, tag="sg")
        nc.scalar.copy(out=sg_local, in_=psumS)
        cc_in = dram.tile([D + 1, K], F32)
        cc_out = dram.tile([N_CORES, D + 1, K], F32)
        nc.gpsimd.dma_start(out=cc_in, in_=sg_local)
        nc.gpsimd.collective_compute(
            "AllGather",
            mybir.AluOpType.bypass,
            replica_groups=[core_ids],
            ins=[cc_in.opt()],
            outs=[cc_out.opt()],
        )
        sg8 = wpool.tile([D + 1, N_CORES, K], F32, tag="sg8")
        ccf = cc_out[0, 0, 0]  # base AP for offset/tensor
        nc.gpsimd.dma_start(
            out=sg8,
            in_=bass.AP(
                tensor=ccf.tensor, offset=ccf.offset,
                ap=[[K, D + 1], [(D + 1) * K, N_CORES], [1, K]],
            ),
        )
        sg4 = wpool.tile([D + 1, 4, K], F32, tag="sg4")
        nc.vector.tensor_add(sg4, sg8[:, :4, :], sg8[:, 4:, :])
        sg2t = wpool.tile([D + 1, 2, K], F32, tag="sg2t")
        nc.vector.tensor_add(sg2t, sg4[:, :2, :], sg4[:, 2:, :])
        sg = wpool.tile([D + 1, K], F32, tag="sg2")
        nc.vector.tensor_tensor(
            sg, sg2t[:, 0, :], sg2t[:, 1, :], mybir.AluOpType.add
        )

        # ---------- ht builds (no AR dependency) ----------
        ht_tiles = []
        for oc in range(n_oc):
            l2, tn = l2_tiles[oc]
            ht = htpool.tile([P, TPAIR * P], BF16, tag="ht")
            nc.vector.tensor_single_scalar(
                ht[:, : tn * P], l2[:, : tn * P], iotacol2,
                mybir.AluOpType.is_equal,
            )
            ht_tiles.append(ht)

        # ---------- stats (Act/Pool/PE only; DVE stays on one-hot work) ----
        psW = psS.tile([K, D + 1], F32, tag="small")
        nc.tensor.transpose(psW, sg, id33)
        W = wpool.tile([K, D + 1], F32, tag="w")  # [S_k | c_k]
        nc.scalar.copy(out=W, in_=psW)
        safec = wpool.tile([K, 1], F32, tag="safec")
        nc.gpsimd.tensor_scalar(
            safec, W[:, D : D + 1], 256.0, 1.0,
            mybir.AluOpType.mult, mybir.AluOpType.max,
        )
        invc = wpool.tile([K, 1], F32, tag="invc")
        nc.vector.reciprocal(invc, safec)
        svp = wpool.tile([K, 1], F32, tag="svp")  # sqrt(1/c)
        nc.scalar.activation(
            out=svp, in_=invc, func=mybir.ActivationFunctionType.Sqrt
        )
        mu = wpool.tile([K, D], F32, tag="mu")
        nc.gpsimd.tensor_mul(mu, W[:, :D], invc.to_broadcast((K, D)))
        # table2 [128, 33] bf16 = [eps - mu | sqrt(1/c) - 1], rows replicated
        table2 = singles.tile([P, D + 1], BF16)
        nc.scalar.activation(
            out=table2[:K, :D], in_=mu,
            func=mybir.ActivationFunctionType.Copy, bias=EPS, scale=-1.0,
        )
        nc.scalar.activation(
            out=table2[:K, D : D + 1], in_=svp,
            func=mybir.ActivationFunctionType.Copy, bias=-1.0 / 256.0,
        )
        psTF = psS.tile([P, D + 1], F32, tag="small")
        nc.tensor.matmul(psTF, idrep, table2[:K, :])
        table2f = singles.tile([P, D + 1], BF16)
        nc.scalar.copy(out=table2f, in_=psTF)

        # ---------- inter + reg losses (Act/Pool/PE) ----------
        mup = wpool.tile([K, D], F32, tag="mup")  # mu + eps
        nc.scalar.activation(
            out=mup, in_=mu, func=mybir.ActivationFunctionType.Copy, bias=EPS
        )
        qsc = wpool.tile([K, D], F32, tag="qsc")
        nc.gpsimd.tensor_mul(qsc, mu, mu)
        q = wpool.tile([K, 1], F32, tag="q")  # ||mu||^2
        nc.vector.tensor_reduce(
            out=q, in_=qsc, axis=mybir.AxisListType.X, op=mybir.AluOpType.add
        )
        qpsc = wpool.tile([K, D], F32, tag="qpsc")
        nc.gpsimd.tensor_mul(qpsc, mup, mup)
        qp = wpool.tile([K, 1], F32, tag="qp")  # ||mu + eps||^2
        nc.vector.tensor_reduce(
            out=qp, in_=qpsc, axis=mybir.AxisListType.X, op=mybir.AluOpType.add
        )
        # pd2[a,b] = qp_a - 2*mup_a.mu_b + q_b via [ -2*mup | qp | 1 ] x [ mu | 1 | q ]
        ab = wpool.tile([K, D + 2], F32, tag="ab")
        nc.scalar.mul(out=ab[:, :D], in_=mup, mul=-2.0)
        nc.scalar.copy(out=ab[:, D : D + 1], in_=qp)
        nc.gpsimd.memset(ab[:, D + 1 : D + 2], 1.0)
        bb = wpool.tile([K, D + 2], F32, tag="bb")
        nc.scalar.copy(out=bb[:, :D], in_=mu)
        nc.gpsimd.memset(bb[:, D : D + 1], 1.0)
        nc.scalar.copy(out=bb[:, D + 1 : D + 2], in_=q)
        psT = psS.tile([D + 2, K], F32, tag="small")
        nc.tensor.transpose(psT, ab, id64)
        atp = wpool.tile([D + 2, K], F32, tag="atp")
        nc.scalar.copy(out=atp, in_=psT)
        psT2 = psS.tile([D + 2, K], F32, tag="small")
        nc.tensor.transpose(psT2, bb, id64)
        btp = wpool.tile([D + 2, K], F32, tag="btp")
        nc.scalar.copy(out=btp, in_=psT2)
        psPD = psS.tile([K, K], F32, tag="small")
        nc.tensor.matmul(psPD, atp, btp)
        pdc = wpool.tile([K, K], F32, tag="pdc")
        nc.vector.tensor_scalar_max(pdc, psPD, 0.0)
        pdist = wpool.tile([K, K], F32, tag="pdist")
        nc.scalar.activation(
            out=pdist, in_=pdc, func=mybir.ActivationFunctionType.Sqrt
        )
        hingeI = wpool.tile([K, K], F32, tag="hingeI")
        nc.scalar.activation(
            out=hingeI, in_=pdist, func=mybir.ActivationFunctionType.Relu,
            bias=float(INTER_MARGIN2), scale=-1.0,
        )
        hm = wpool.tile([K, K], F32, tag="hm")
        nc.gpsimd.tensor_mul(hm, hingeI, eyeneg)
        hm2 = wpool.tile([K, K], F32, tag="hm2")
        nc.gpsimd.tensor_mul(hm2, hm, hm)
        interp = wpool.tile([K, 1], F32, tag="interp")
        nc.vector.tensor_reduce(
            out=interp, in_=hm2, axis=mybir.AxisListType.X,
            op=mybir.AluOpType.add,
        )
        sqp = wpool.tile([K, 1], F32, tag="sqp")  # ||mu + eps||
        nc.scalar.activation(
            out=sqp, in_=qp, func=mybir.ActivationFunctionType.Sqrt
        )
        cat2 = wpool.tile([K, 2], F32, tag="cat2")
        nc.scalar.copy(out=cat2[:, 0:1], in_=interp)
        nc.scalar.copy(out=cat2[:, 1:2], in_=sqp)
        ones64 = singles.tile([K, 1], F32)
        nc.gpsimd.memset(ones64, 1.0)
        psIR = psS.tile([1, 2], F32, tag="small")
        nc.tensor.matmul(psIR, ones64, cat2)
        ir = wpool.tile([1, 2], F32, tag="ir")  # [inter_sum, reg_sum]
        nc.scalar.copy(out=ir, in_=psIR)

        # ---------- pass 2: gather + diff in PSUM, square, fold-reduce ------
        fin_oc = sorted(set(
            [n_oc - 1] + [max(0, (n_oc * (q + 1)) // 4 - 1) for q in range(3)]
        ))
        fin_base = []
        prev = 0
        for oc_ in fin_oc:
            fin_base.append(prev)
            pc_ = min(TPAIR, npair - oc_ * TPAIR)
            prev = min(oc_ * TPAIR * 2 + pc_ * 2, tpc)
        for oc in range(n_oc):
            ht = ht_tiles[oc]
            pc = min(TPAIR, npair - oc * TPAIR)
            cbase = oc * TPAIR * 2        # first global tile of this chunk
            ctn = min(pc * 2, tpc - cbase)  # tiles in this chunk
            # one sq tile per ht chunk (up to 56 tiles), 4 PSUM groups
            sq = sqpool.tile([P, TPAIR * 2, D + 1], BF16, tag="sq")
            for g in range(math.ceil(pc / (JMG // 2))):
                p0 = g * (JMG // 2)
                pn = min(JMG // 2, pc - p0)
                jbase = (oc * TPAIR + p0) * 2  # first global tile of group
                nt = min(pn * 2, tpc - jbase)
                psD = psMg.tile([P, JMG, D + 1], F32, tag="psd")
                for lp in range(pn):
                    for half in range(2):
                        t = lp * 2 + half
                        if t >= nt:
                            break
                        colp = p0 + lp
                        nc.tensor.matmul(
                            psD[:, t, :],
                            ht[64 * half : 64 * (half + 1),
                               colp * P : (colp + 1) * P],
                            table2f[64 * half : 64 * (half + 1), :],
                            start=True, stop=False,
                        )
                        nc.tensor.matmul(
                            psD[:, t, :], id128,
                            xe[:, cbase + p0 * 2 + t, :],
                            start=False, stop=True,
                        )
                t0 = p0 * 2
                if dve_sq_every and (oc * 4 + g) % dve_sq_every == dve_sq_every - 1:
                    # DVE square: PSUM copy then bf16 self-mult (one PSUM
                    # input per instruction as required by hardware)
                    cpy = sqpool.tile([P, JMG, D + 1], BF16, tag="cpy")
                    nc.vector.tensor_scalar_add(
                        cpy[:, :nt, :], psD[:, :nt, :], 0.0
                    )
                    nc.vector.tensor_mul(
                        sq[:, t0 : t0 + nt, :], cpy[:, :nt, :], cpy[:, :nt, :]
                    )
                else:
                    nc.scalar.activation(
                        out=sq[:, t0 : t0 + nt, :], in_=psD[:, :nt, :],
                        func=mybir.ActivationFunctionType.Square,
                    )
            # bf16 fold-tree reduce over D (2x DVE mode); f1 of every other
            # chunk goes to Pool to offload DVE
            with nc.allow_low_precision(reason="bf16 partial sums of d2"):
                f1 = sqpool.tile([P, TPAIR * 2, 16], BF16, tag="f1")
                f1eng = nc.vector if (f1_dve_every and oc % f1_dve_every == f1_dve_every - 1) else nc.gpsimd
                f1eng.tensor_add(
                    f1[:, :ctn, :], sq[:, :ctn, 0:16], sq[:, :ctn, 16:32]
                )
                f2 = sqpool.tile([P, TPAIR * 2, 8], BF16, tag="f2")
                nc.vector.tensor_add(
                    f2[:, :ctn, :], f1[:, :ctn, 0:8], f1[:, :ctn, 8:16]
                )
                f3 = sqpool.tile([P, TPAIR * 2, 4], BF16, tag="f3")
                nc.vector.tensor_add(
                    f3[:, :ctn, :], f2[:, :ctn, 0:4], f2[:, :ctn, 4:8]
                )
                f4 = sqpool.tile([P, TPAIR * 2, 2], BF16, tag="f4")
                nc.vector.tensor_add(
                    f4[:, :ctn, :], f3[:, :ctn, 0:2], f3[:, :ctn, 2:4]
                )
            nc.vector.tensor_tensor(
                d2all[:, cbase : cbase + ctn],
                f4[:, :ctn, 0], f4[:, :ctn, 1], mybir.AluOpType.add,
            )
            nc.gpsimd.tensor_scalar_add(
                invc_all[:, cbase : cbase + ctn], sq[:, :ctn, D], 0.0
            )
            # quarter-granularity finals (keeps the serial tail short
            # without flooding Act with per-chunk overhead):
            # dist = sqrt(d2); h = relu(dist - 1.5); acc_q = sum h^2 * invc
            if oc in fin_oc:
                qi = fin_oc.index(oc)
                b0 = fin_base[qi]
                b1 = cbase + ctn
                dsl = d2all[:, b0:b1]
                nc.scalar.activation(
                    out=dsl, in_=dsl,
                    func=mybir.ActivationFunctionType.Sqrt,
                )
                nc.scalar.activation(
                    out=dsl, in_=dsl,
                    func=mybir.ActivationFunctionType.Relu, bias=margneg,
                )
                hsl = hh[:, b0:b1]
                nc.vector.tensor_mul(hsl, dsl, dsl)
                wsl = hhw[:, b0:b1]
                nc.vector.tensor_mul(wsl, hsl, invc_all[:, b0:b1])
                nc.vector.tensor_reduce(
                    out=rsacc[:, qi : qi + 1], in_=wsl,
                    axis=mybir.AxisListType.X, op=mybir.AluOpType.add,
                )

        # ---------- finals: reduce per-quarter partials ----------
        rowsum = singles.tile([P, 1], F32)
        nc.vector.tensor_reduce(
            out=rowsum, in_=rsacc[:, : len(fin_oc)],
            axis=mybir.AxisListType.X, op=mybir.AluOpType.add,
        )
        ones128 = singles.tile([P, 1], F32)
        nc.gpsimd.memset(ones128, 1.0)
        psL = psS.tile([1, 1], F32, tag="small")
        nc.tensor.matmul(psL, rowsum, ones128)
        tot = wpool.tile([1, 3], F32, tag="tot")
        nc.scalar.copy(out=tot[:, 0:1], in_=psL)
        nc.scalar.copy(out=tot[:, 1:3], in_=ir)
        nc.sync.dma_start(out=out_d, in_=tot[0:1, :])

    nc.compile()
    return nc


_NC_CACHE = {}


def _get_program(tpc):
    if tpc not in _NC_CACHE:
        _NC_CACHE[tpc] = build_program(tpc)
    return _NC_CACHE[tpc]


def kernel(features, labels, num_clusters):
    features = np.asarray(features)
    labels = np.asarray(labels)
    n_total = features.shape[0]
    n_core = n_total // N_CORES
    tpc = math.ceil(n_core / P)
    nc = _get_program(tpc)
    in_maps = _host_prep(features, labels, tpc)
    res = run_bass_kernel_spmd(nc, in_maps, list(range(N_CORES)))
    intra_sum = sum(float(res.results[c]["out"][0]) for c in range(N_CORES))
    inter_sum = float(res.results[0]["out"][1])
    reg_sum = float(res.results[0]["out"][2])
    total = (
        intra_sum / K
        + inter_sum / (K * (K - 1))
        + 0.001 * reg_sum / K
    )
    return np.float32(total)


# revision 98
# speedup vs baseline: 1.0190x; 1.0166x over previous
# kernel.py — DiscriminativeLoss on 8 TRN2 NeuronCores (Bass/Tile, SPMD).
#
# Math (matches reference):
#   counts_k = #{i: l_i = k};  S_k = sum_{i in k} x_i;  mu_k = S_k / max(c_k, 1)
#   intra = (1/K) * sum_i invc_{l_i} * relu(||x_i - mu_{l_i} + eps|| - 1.5)^2
#   inter = sum_{a != b} relu(1 - ||(mu_a + eps) - mu_b||)^2 / (K*(K-1))
#   reg   = (1/K) * sum_k ||mu_k + eps||
#   total = intra + inter + 0.001 * reg
#
# V2 design (engine-balanced, cost-model driven):
#   pass 1: one-hot H2 [P, K, jn] via DVE/Pool tensor_tensor is_equal (2x
#     mode: all operands 2-byte packed SBUF); PE matmul lhsT=Xe [128,33],
#     rhs=H2[:, :, j] accumulates S^T = [S | counts] in PSUM [33, 64].
#   AllReduce [33, 64]; stats (mu, 1/c, sqrt(1/c), inter/reg losses) on
#     Act/Pool/PE only, keeping DVE free.
#   pass 2: paired transposed one-hot ht [128, 128] per tile-pair (A on
#     partitions 0-63, B on 64-127) built from broadcast-DMA'd labels via
#     DVE tensor_single_scalar is_equal (4x mode). Per tile, TWO accumulating
#     matmuls produce diff = x - (mu - eps) directly in PSUM:
#       psD  = ht_half^T @ [eps - mu | sqrt(1/c) - 1]   (gather, negated)
#       psD += I_128    @ Xe_tile                        ([x | 1])
#     -> psD = [x - mu + eps | sqrt(1/c)].
#   Act Square psD -> sq bf16 (col 32 squares to 1/c); DVE/Pool tensor_reduce
#     over D -> d2; DVE copies col 32 -> invc_all.
#   finals: dist=sqrt(d2), h=relu(dist-1.5) on Act; intra partial
#     sum_i h^2 * invc via DVE mults + tensor_tensor_reduce + PE ones-matmul.
import math
import numpy as np
from contextlib import ExitStack

import concourse.bass as bass
import concourse.bacc as bacc
import concourse.tile as tile
import concourse.mybir as mybir
from concourse.bass_utils import run_bass_kernel_spmd

F32 = mybir.dt.float32
BF16 = mybir.dt.bfloat16
I16 = mybir.dt.int16

N_CORES = 8
K = 64
D = 32
P = 128
EPS = 1e-8
PAD_LABEL = 999  # never matches any one-hot column

INTRA_MARGIN = 1.5
INTER_MARGIN2 = 1.0  # 2 * 0.5

J1 = 20      # pass-1 chunk width (tiles)
NACT = 0     # pass-1 tiles whose one-hot is built on Act (PE rank-3 + relu)
TPAIR = 28   # tile-pairs per ht chunk (4 gather groups of 7 pairs)
JMG = 14     # tiles per PSUM gather group (7 pairs)


def _host_prep(features, labels, tpc):
    """Shard + relayout on host. Returns per-core input dicts."""
    n_total = features.shape[0]
    n_core = n_total // N_CORES
    n_pad = P * tpc
    npair = (tpc + 1) // 2
    import ml_dtypes

    in_maps = []
    for c in range(N_CORES):
        f = np.asarray(features[c * n_core : (c + 1) * n_core], dtype=np.float32)
        l = np.asarray(labels[c * n_core : (c + 1) * n_core], dtype=np.int64)
        if n_pad > n_core:
            f = np.concatenate([f, np.zeros((n_pad - n_core, D), np.float32)], axis=0)
            l = np.concatenate([l, np.full((n_pad - n_core,), PAD_LABEL, np.int64)])
        # Xe: [P, tpc, 33] bf16, col 32 = 1/256 (exact in bf16; keeps the
        # sqrt(1/c) gather free of bf16 cancellation); point i = (i%P, i//P)
        xe = np.full((n_pad, D + 1), 1.0 / 256.0, np.float32)
        xe[:, :D] = f
        xe = xe.reshape(P, tpc, D + 1).astype(ml_dtypes.bfloat16)
        lpj = l.reshape(P, tpc)  # [point-in-tile, tile]
        l_pm = lpj.astype(np.int16)
        # paired label broadcast for ht, fully materialized on host:
        # [128, npair*128] int16, rows 0-63 = labels of tile 2jj, rows
        # 64-127 = labels of tile 2jj+1 (one contiguous DMA per chunk)
        ltm = lpj.T.astype(np.int16)  # [tpc, P]
        l_tma = np.full((npair, P), PAD_LABEL, np.int16)
        l_tmb = np.full((npair, P), PAD_LABEL, np.int16)
        l_tma[:] = ltm[0::2]
        nb = tpc // 2
        l_tmb[:nb] = ltm[1::2]
        l2full = np.empty((P, npair * P), np.int16)
        l2full[:K] = np.broadcast_to(
            l_tma.reshape(1, npair * P), (K, npair * P)
        )
        l2full[K:] = np.broadcast_to(
            l_tmb.reshape(1, npair * P), (K, npair * P)
        )
        # iotarep [P, K, J1] int16: value k at [:, k, :]
        iotarep = np.tile(
            np.arange(K, dtype=np.int16)[None, :, None], (P, 1, J1)
        )
        # rank-2 one-hot operands for the Act-built pass-1 tail (last NACT
        # tiles): per tail tile r, rows [1; l] live at partitions
        # 2*(r%64), +1, column band r//64 (PAD remapped to 100; all values
        # exact in bf16). PE gives (k - l) exactly; Act Square + Relu(1-x)
        # recover the one-hot.
        n_act_t = min(NACT, tpc)
        lsm = np.where(lpj == PAD_LABEL, 100, lpj).T.astype(np.int64)  # [tpc, P]
        nbands = max(1, math.ceil(n_act_t / 3))
        lr3h = np.zeros((P, nbands * P), np.float32)
        for r in range(n_act_t):
            s, b = r % 3, r // 3  # slot partition offsets 0/32/64 only
            lr3h[32 * s, b * P : (b + 1) * P] = 1.0
            lr3h[32 * s + 1, b * P : (b + 1) * P] = lsm[r]
        lr3h = lr3h.astype(ml_dtypes.bfloat16)
        kv = np.arange(K, dtype=np.float32)
        kvecrep = np.zeros((P, K), np.float32)
        for s in range(3):
            kvecrep[32 * s] = kv
            kvecrep[32 * s + 1] = -1.0
        kvecrep = kvecrep.astype(ml_dtypes.bfloat16)  # [128, K]
        in_maps.append(
            {
                "xe": np.ascontiguousarray(xe),
                "labels_pm": np.ascontiguousarray(l_pm),
                "l2full": l2full,
                "iotarep": np.ascontiguousarray(iotarep),
                "lr3h": np.ascontiguousarray(lr3h),
                "kvecrep": np.ascontiguousarray(kvecrep),
                "iotacol2": np.concatenate(
                    [np.arange(K), np.arange(K)]
                ).astype(np.float32).reshape(P, 1),
                "id128": np.eye(P, dtype=ml_dtypes.bfloat16),
                "idrep": np.ascontiguousarray(np.hstack(
                    [np.eye(K), np.eye(K)]).astype(ml_dtypes.bfloat16)),
                "id33": np.eye(D + 1, dtype=np.float32),
                "id64": np.eye(K, dtype=np.float32),
                "eyeneg": (1.0 - np.eye(K, dtype=np.float32)).astype(
                    ml_dtypes.bfloat16
                ),
            }
        )
    return in_maps


def build_program(tpc, dve_sq_every=6, f1_dve_every=0, ht_bufs=7, l2_bufs=3, mg_bufs=4):
    """Build the SPMD Bass program. tpc = tiles per core."""
    nc = bacc.Bacc(
        "TRN2", target_bir_lowering=False, debug=False, num_devices=N_CORES
    )
    core_ids = list(range(N_CORES))
    npair = (tpc + 1) // 2

    xe_d = nc.dram_tensor("xe", [P, tpc, D + 1], BF16, kind="ExternalInput").ap()
    lpm_d = nc.dram_tensor("labels_pm", [P, tpc], I16, kind="ExternalInput").ap()
    l2f_d = nc.dram_tensor("l2full", [P, npair * P], I16, kind="ExternalInput").ap()
    iotarep_d = nc.dram_tensor("iotarep", [P, K, J1], I16, kind="ExternalInput").ap()
    n_act_tiles = min(NACT, tpc)
    n_dve_tiles = tpc - n_act_tiles
    nbands = max(1, math.ceil(n_act_tiles / 3))
    lr3h_d = nc.dram_tensor("lr3h", [P, nbands * P], BF16, kind="ExternalInput").ap()
    kvecrep_d = nc.dram_tensor("kvecrep", [P, K], BF16, kind="ExternalInput").ap()
    iotacol2_d = nc.dram_tensor("iotacol2", [P, 1], F32, kind="ExternalInput").ap()
    id128_d = nc.dram_tensor("id128", [P, P], BF16, kind="ExternalInput").ap()
    idrep_d = nc.dram_tensor("idrep", [K, P], BF16, kind="ExternalInput").ap()
    id33_d = nc.dram_tensor("id33", [D + 1, D + 1], F32, kind="ExternalInput").ap()
    id64_d = nc.dram_tensor("id64", [K, K], F32, kind="ExternalInput").ap()
    eyeneg_d = nc.dram_tensor("eyeneg", [K, K], BF16, kind="ExternalInput").ap()
    out_d = nc.dram_tensor("out", [3], F32, kind="ExternalOutput").ap()

    n_chunks1 = math.ceil(tpc / J1)
    n_oc = math.ceil(npair / TPAIR)

    with tile.TileContext(nc, num_cores=N_CORES) as tc, ExitStack() as ctx:
        singles = ctx.enter_context(tc.tile_pool(name="singles", bufs=1))
        xpool = ctx.enter_context(tc.tile_pool(name="xpool", bufs=1))
        h2pool = ctx.enter_context(tc.tile_pool(name="h2pool", bufs=3))
        hqpool = ctx.enter_context(tc.tile_pool(name="hqpool", bufs=2))
        l2pool = ctx.enter_context(tc.tile_pool(name="l2pool", bufs=l2_bufs))
        htpool = ctx.enter_context(tc.tile_pool(name="htpool", bufs=ht_bufs))
        sqpool = ctx.enter_context(tc.tile_pool(name="sqpool", bufs=3))  # sq/f1..f4/cpy tags
        wpool = ctx.enter_context(tc.tile_pool(name="wpool", bufs=2))
        psA = ctx.enter_context(tc.tile_pool(name="psA", bufs=1, space="PSUM"))
        psQp = ctx.enter_context(tc.tile_pool(name="psQp", bufs=1, space="PSUM"))
        psMg = ctx.enter_context(tc.tile_pool(name="psMg", bufs=mg_bufs, space="PSUM"))
        psS = ctx.enter_context(tc.tile_pool(name="psS", bufs=2, space="PSUM"))
        dram = ctx.enter_context(tc.tile_pool(name="dram", bufs=2, space="DRAM"))

        # ---------- constants ----------
        lpm = singles.tile([P, tpc], I16)
        nc.sync.dma_start(out=lpm, in_=lpm_d)
        iotarep = singles.tile([P, K, J1], I16)
        nc.sync.dma_start(out=iotarep, in_=iotarep_d)
        iotacol2 = singles.tile([P, 1], F32)
        nc.sync.dma_start(out=iotacol2, in_=iotacol2_d)
        id128 = singles.tile([P, P], BF16)
        nc.sync.dma_start(out=id128, in_=id128_d)
        idrep = singles.tile([K, P], BF16)
        nc.sync.dma_start(out=idrep, in_=idrep_d)
        id33 = singles.tile([D + 1, D + 1], F32)
        nc.sync.dma_start(out=id33, in_=id33_d)
        id64 = singles.tile([K, K], F32)
        nc.sync.dma_start(out=id64, in_=id64_d)
        eyeneg = singles.tile([K, K], BF16)
        nc.sync.dma_start(out=eyeneg, in_=eyeneg_d)
        d2all = singles.tile([P, tpc], F32)
        invc_all = singles.tile([P, tpc], BF16)
        hh = singles.tile([P, tpc], F32)
        hhw = singles.tile([P, tpc], F32)
        rsacc = singles.tile([P, n_oc], F32)
        margneg = singles.tile([P, 1], F32)
        nc.gpsimd.memset(margneg, -float(INTRA_MARGIN))

        # l2 chunk DMA helper (host-materialized paired label broadcast)
        def issue_l2(oc):
            t0 = oc * TPAIR
            tn = min(TPAIR, npair - t0)
            l2 = l2pool.tile([P, TPAIR * P], I16, tag="l2")
            nc.sync.dma_start(
                out=l2[:, : tn * P], in_=l2f_d[:, t0 * P : (t0 + tn) * P]
            )
            return l2, tn

        # ---------- pass 1: segment sums ----------
        # xe DMAs issued first so pass-1 is never starved by the (large)
        # l2 broadcast transfers; l2 chunks are issued after so the
        # collective is not queued behind them on the DMA engines.
        # The LAST n_act_tiles tiles use an Act-engine one-hot instead of
        # DVE: PE rank-3 matmul gives (k - l)^2 in PSUM, Act relu(1 - x)
        # turns it into the one-hot (Act is otherwise idle before the
        # collective; this shortens the DVE-bound pass-1 phase).
        lr3 = singles.tile([P, nbands * P], BF16)
        nc.sync.dma_start(out=lr3, in_=lr3h_d)
        kvec = singles.tile([P, K], BF16)
        nc.sync.dma_start(out=kvec, in_=kvecrep_d)
        psumS = psA.tile([D + 1, K], F32)
        l2_tiles = []
        lc = 0
        t_done = 0
        # Act-built one-hot groups (7 tiles per PSUM bank), interleaved
        # among the DVE-built chunks so the PE queue never stalls long on
        # the PE->Act->PE round trip; Act is otherwise idle pre-collective.
        JQ = 7
        n_qgroups = math.ceil(n_act_tiles / JQ)

        def emit_act_group(qg):
            global_t = globals()  # noqa - placeholder
        def act_group(qg, t_done):
            q0 = qg * JQ
            qn = min(JQ, n_act_tiles - q0)
            nc.sync.dma_start(
                out=xe[:, q0 : q0 + qn, :], in_=xe_d[:, q0 : q0 + qn, :]
            )
            psQ = psQp.tile([P, JQ, K], F32, tag="psq")
            for t in range(qn):
                r = q0 + t
                s, b = r % 3, r // 3
                nc.tensor.matmul(
                    psQ[:, t, :],
                    lr3[32 * s : 32 * s + 2, b * P : (b + 1) * P],
                    kvec[32 * s : 32 * s + 2, :],
                    start=True, stop=True,
                )
            h2sq = hqpool.tile([P, JQ, K], BF16, tag="h2sq")
            nc.scalar.activation(
                out=h2sq[:, :qn, :], in_=psQ[:, :qn, :],
                func=mybir.ActivationFunctionType.Square,
            )
            h2a = hqpool.tile([P, JQ, K], BF16, tag="h2a")
            nc.scalar.activation(
                out=h2a[:, :qn, :], in_=h2sq[:, :qn, :],
                func=mybir.ActivationFunctionType.Relu, bias=1.0, scale=-1.0,
            )
            for t in range(qn):
                nc.tensor.matmul(
                    psumS,
                    xe[:, q0 + t, :],
                    h2a[:, t, :],
                    start=(t_done == 0),
                    stop=(t_done == tpc - 1),
                )
                t_done += 1
            return t_done

        # DVE-built chunks; xe is streamed through a rolling pool (the
        # full-size resident copy is gone — pass 2 re-streams its own xe
        # chunks during the otherwise idle collective window, freeing
        # ~64KB of SBUF for a much deeper ht pool)
        n_chunks1d = math.ceil(n_dve_tiles / J1)
        qg_next = 0
        xe = xpool.tile([P, tpc, D + 1], BF16)
        for c in range(n_chunks1d):
            j0 = n_act_tiles + c * J1
            jn = min(J1, tpc - j0)
            nc.sync.dma_start(
                out=xe[:, j0 : j0 + jn, :], in_=xe_d[:, j0 : j0 + jn, :]
            )
            h2 = h2pool.tile([P, K, J1], BF16, tag="h2")
            nc.vector.tensor_tensor(
                h2[:, :, :jn],
                lpm[:, None, j0 : j0 + jn].to_broadcast((P, K, jn)),
                iotarep[:, :, :jn],
                mybir.AluOpType.is_equal,
            )
            for j in range(jn):
                nc.tensor.matmul(
                    psumS,
                    xe[:, j0 + j, :],
                    h2[:, :, j],
                    start=(t_done == 0),
                    stop=(t_done == tpc - 1),
                )
                t_done += 1
        # l2 label chunks stream right behind xe on the DMA engines
        while lc < n_oc:
            l2_tiles.append(issue_l2(lc))
            lc += 1
        # ---------- AllGather segment sums + local reduce ----------
        # (AllGather avoids the cost model's 1.875x AllReduce penalty; the
        #  8-way sum is 3 cheap tree adds done locally)
        sg_local = wpool.tile([D + 1, K], BF16, tag="sg")
        nc.scalar.copy(out=sg_local, in_=psumS)
        cc_in = dram.tile([D + 1, K], BF16)
        cc_out = dram.tile([N_CORES, D + 1, K], BF16)
        nc.gpsimd.dma_start(out=cc_in, in_=sg_local)
        nc.gpsimd.collective_compute(
            "AllGather",
            mybir.AluOpType.bypass,
            replica_groups=[core_ids],
            ins=[cc_in.opt()],
            outs=[cc_out.opt()],
        )
        sg8 = wpool.tile([D + 1, N_CORES, K], BF16, tag="sg8")
        ccf = cc_out[0, 0, 0]  # base AP for offset/tensor
        nc.gpsimd.dma_start(
            out=sg8,
            in_=bass.AP(
                tensor=ccf.tensor, offset=ccf.offset,
                ap=[[K, D + 1], [(D + 1) * K, N_CORES], [1, K]],
            ),
        )
        with nc.allow_low_precision(reason="bf16 cross-core segment sums"):
            sg4 = wpool.tile([D + 1, 4, K], BF16, tag="sg4")
            nc.vector.tensor_add(sg4, sg8[:, :4, :], sg8[:, 4:, :])
            sg2t = wpool.tile([D + 1, 2, K], BF16, tag="sg2t")
            nc.vector.tensor_add(sg2t, sg4[:, :2, :], sg4[:, 2:, :])
        sg = wpool.tile([D + 1, K], F32, tag="sg2")
        nc.vector.tensor_tensor(
            sg, sg2t[:, 0, :], sg2t[:, 1, :], mybir.AluOpType.add
        )

        # ---------- ht builds (no AR dependency) ----------
        ht_tiles = []
        for oc in range(n_oc):
            l2, tn = l2_tiles[oc]
            ht = htpool.tile([P, TPAIR * P], BF16, tag="ht")
            nc.vector.tensor_single_scalar(
                ht[:, : tn * P], l2[:, : tn * P], iotacol2,
                mybir.AluOpType.is_equal,
            )
            ht_tiles.append(ht)

        # ---------- stats (Act/Pool/PE only; DVE stays on one-hot work) ----
        psW = psS.tile([K, D + 1], F32, tag="small")
        nc.tensor.transpose(psW, sg, id33)
        W = wpool.tile([K, D + 1], F32, tag="w")  # [S_k | c_k]
        nc.scalar.copy(out=W, in_=psW)
        safec = wpool.tile([K, 1], F32, tag="safec")
        nc.gpsimd.tensor_scalar(
            safec, W[:, D : D + 1], 256.0, 1.0,
            mybir.AluOpType.mult, mybir.AluOpType.max,
        )
        invc = wpool.tile([K, 1], F32, tag="invc")
        nc.vector.reciprocal(invc, safec)
        svp = wpool.tile([K, 1], F32, tag="svp")  # sqrt(1/c)
        nc.scalar.activation(
            out=svp, in_=invc, func=mybir.ActivationFunctionType.Sqrt
        )
        mu = wpool.tile([K, D], F32, tag="mu")
        nc.gpsimd.tensor_mul(mu, W[:, :D], invc.to_broadcast((K, D)))
        # table2 [128, 33] bf16 = [eps - mu | sqrt(1/c) - 1], rows replicated
        table2 = singles.tile([P, D + 1], BF16)
        nc.scalar.activation(
            out=table2[:K, :D], in_=mu,
            func=mybir.ActivationFunctionType.Copy, bias=EPS, scale=-1.0,
        )
        nc.scalar.activation(
            out=table2[:K, D : D + 1], in_=svp,
            func=mybir.ActivationFunctionType.Copy, bias=-1.0 / 256.0,
        )
        psTF = psS.tile([P, D + 1], F32, tag="small")
        nc.tensor.matmul(psTF, idrep, table2[:K, :])
        table2f = singles.tile([P, D + 1], BF16)
        nc.scalar.copy(out=table2f, in_=psTF)

        # ---------- inter + reg losses (Act/Pool/PE) ----------
        mup = wpool.tile([K, D], F32, tag="mup")  # mu + eps
        nc.scalar.activation(
            out=mup, in_=mu, func=mybir.ActivationFunctionType.Copy, bias=EPS
        )
        qsc = wpool.tile([K, D], F32, tag="qsc")
        nc.gpsimd.tensor_mul(qsc, mu, mu)
        q = wpool.tile([K, 1], F32, tag="q")  # ||mu||^2
        nc.vector.tensor_reduce(
            out=q, in_=qsc, axis=mybir.AxisListType.X, op=mybir.AluOpType.add
        )
        qpsc = wpool.tile([K, D], F32, tag="qpsc")
        nc.gpsimd.tensor_mul(qpsc, mup, mup)
        qp = wpool.tile([K, 1], F32, tag="qp")  # ||mu + eps||^2
        nc.vector.tensor_reduce(
            out=qp, in_=qpsc, axis=mybir.AxisListType.X, op=mybir.AluOpType.add
        )
        # pd2[a,b] = qp_a - 2*mup_a.mu_b + q_b via [ -2*mup | qp | 1 ] x [ mu | 1 | q ]
        ab = wpool.tile([K, D + 2], F32, tag="ab")
        nc.scalar.mul(out=ab[:, :D], in_=mup, mul=-2.0)
        nc.scalar.copy(out=ab[:, D : D + 1], in_=qp)
        nc.gpsimd.memset(ab[:, D + 1 : D + 2], 1.0)
        bb = wpool.tile([K, D + 2], F32, tag="bb")
        nc.scalar.copy(out=bb[:, :D], in_=mu)
        nc.gpsimd.memset(bb[:, D : D + 1], 1.0)
        nc.scalar.copy(out=bb[:, D + 1 : D + 2], in_=q)
        psT = psS.tile([D + 2, K], F32, tag="small")
        nc.tensor.transpose(psT, ab, id64)
        atp = wpool.tile([D + 2, K], F32, tag="atp")
        nc.scalar.copy(out=atp, in_=psT)
        psT2 = psS.tile([D + 2, K], F32, tag="small")
        nc.tensor.transpose(psT2, bb, id64)
        btp = wpool.tile([D + 2, K], F32, tag="btp")
        nc.scalar.copy(out=btp, in_=psT2)
        psPD = psS.tile([K, K], F32, tag="small")
        nc.tensor.matmul(psPD, atp, btp)
        pdc = wpool.tile([K, K], F32, tag="pdc")
        nc.vector.tensor_scalar_max(pdc, psPD, 0.0)
        pdist = wpool.tile([K, K], F32, tag="pdist")
        nc.scalar.activation(
            out=pdist, in_=pdc, func=mybir.ActivationFunctionType.Sqrt
        )
        hingeI = wpool.tile([K, K], F32, tag="hingeI")
        nc.scalar.activation(
            out=hingeI, in_=pdist, func=mybir.ActivationFunctionType.Relu,
            bias=float(INTER_MARGIN2), scale=-1.0,
        )
        hm = wpool.tile([K, K], F32, tag="hm")
        nc.gpsimd.tensor_mul(hm, hingeI, eyeneg)
        hm2 = wpool.tile([K, K], F32, tag="hm2")
        nc.gpsimd.tensor_mul(hm2, hm, hm)
        interp = wpool.tile([K, 1], F32, tag="interp")
        nc.vector.tensor_reduce(
            out=interp, in_=hm2, axis=mybir.AxisListType.X,
            op=mybir.AluOpType.add,
        )
        sqp = wpool.tile([K, 1], F32, tag="sqp")  # ||mu + eps||
        nc.scalar.activation(
            out=sqp, in_=qp, func=mybir.ActivationFunctionType.Sqrt
        )
        cat2 = wpool.tile([K, 2], F32, tag="cat2")
        nc.scalar.copy(out=cat2[:, 0:1], in_=interp)
        nc.scalar.copy(out=cat2[:, 1:2], in_=sqp)
        ones64 = singles.tile([K, 1], F32)
        nc.gpsimd.memset(ones64, 1.0)
        psIR = psS.tile([1, 2], F32, tag="small")
        nc.tensor.matmul(psIR, ones64, cat2)
        ir = wpool.tile([1, 2], F32, tag="ir")  # [inter_sum, reg_sum]
        nc.scalar.copy(out=ir, in_=psIR)

        # ---------- pass 2: gather + diff in PSUM, square, fold-reduce ------
        fin_oc = sorted(set(
            [n_oc - 1] + [max(0, (n_oc * (q + 1)) // 4 - 1) for q in range(3)]
        ))
        fin_base = []
        prev = 0
        for oc_ in fin_oc:
            fin_base.append(prev)
            pc_ = min(TPAIR, npair - oc_ * TPAIR)
            prev = min(oc_ * TPAIR * 2 + pc_ * 2, tpc)
        for oc in range(n_oc):
            ht = ht_tiles[oc]
            pc = min(TPAIR, npair - oc * TPAIR)
            cbase = oc * TPAIR * 2        # first global tile of this chunk
            ctn = min(pc * 2, tpc - cbase)  # tiles in this chunk
            # one sq tile per ht chunk (up to 56 tiles), 4 PSUM groups
            sq = sqpool.tile([P, TPAIR * 2, D + 1], BF16, tag="sq")
            for g in range(math.ceil(pc / (JMG // 2))):
                p0 = g * (JMG // 2)
                pn = min(JMG // 2, pc - p0)
                jbase = (oc * TPAIR + p0) * 2  # first global tile of group
                nt = min(pn * 2, tpc - jbase)
                psD = psMg.tile([P, JMG, D + 1], F32, tag="psd")
                for lp in range(pn):
                    for half in range(2):
                        t = lp * 2 + half
                        if t >= nt:
                            break
                        colp = p0 + lp
                        nc.tensor.matmul(
                            psD[:, t, :],
                            ht[64 * half : 64 * (half + 1),
                               colp * P : (colp + 1) * P],
                            table2f[64 * half : 64 * (half + 1), :],
                            start=True, stop=False,
                        )
                        nc.tensor.matmul(
                            psD[:, t, :], id128,
                            xe[:, cbase + p0 * 2 + t, :],
                            start=False, stop=True,
                        )
                t0 = p0 * 2
                if dve_sq_every and (oc * 4 + g) % dve_sq_every == dve_sq_every - 1:
                    # DVE square: PSUM copy then bf16 self-mult (one PSUM
                    # input per instruction as required by hardware)
                    cpy = sqpool.tile([P, JMG, D + 1], BF16, tag="cpy")
                    nc.vector.tensor_scalar_add(
                        cpy[:, :nt, :], psD[:, :nt, :], 0.0
                    )
                    nc.vector.tensor_mul(
                        sq[:, t0 : t0 + nt, :], cpy[:, :nt, :], cpy[:, :nt, :]
                    )
                else:
                    nc.scalar.activation(
                        out=sq[:, t0 : t0 + nt, :], in_=psD[:, :nt, :],
                        func=mybir.ActivationFunctionType.Square,
                    )
            # bf16 fold-tree reduce over D (2x DVE mode); f1 of every other
            # chunk goes to Pool to offload DVE
            with nc.allow_low_precision(reason="bf16 partial sums of d2"):
                f1 = sqpool.tile([P, TPAIR * 2, 16], BF16, tag="f1")
                f1eng = nc.vector if (f1_dve_every and oc % f1_dve_every == f1_dve_every - 1) else nc.gpsimd
                f1eng.tensor_add(
                    f1[:, :ctn, :], sq[:, :ctn, 0:16], sq[:, :ctn, 16:32]
                )
                f2 = sqpool.tile([P, TPAIR * 2, 8], BF16, tag="f2")
                nc.vector.tensor_add(
                    f2[:, :ctn, :], f1[:, :ctn, 0:8], f1[:, :ctn, 8:16]
                )
                f3 = sqpool.tile([P, TPAIR * 2, 4], BF16, tag="f3")
                nc.vector.tensor_add(
                    f3[:, :ctn, :], f2[:, :ctn, 0:4], f2[:, :ctn, 4:8]
                )
                f4 = sqpool.tile([P, TPAIR * 2, 2], BF16, tag="f4")
                nc.vector.tensor_add(
                    f4[:, :ctn, :], f3[:, :ctn, 0:2], f3[:, :ctn, 2:4]
                )
            nc.vector.tensor_tensor(
                d2all[:, cbase : cbase + ctn],
                f4[:, :ctn, 0], f4[:, :ctn, 1], mybir.AluOpType.add,
            )
            nc.gpsimd.tensor_scalar_add(
                invc_all[:, cbase : cbase + ctn], sq[:, :ctn, D], 0.0
            )
            # quarter-granularity finals (keeps the serial tail short
            # without flooding Act with per-chunk overhead):
            # dist = sqrt(d2); h = relu(dist - 1.5); acc_q = sum h^2 * invc
            if oc in fin_oc:
                qi = fin_oc.index(oc)
                b0 = fin_base[qi]
                b1 = cbase + ctn
                dsl = d2all[:, b0:b1]
                nc.scalar.activation(
                    out=dsl, in_=dsl,
                    func=mybir.ActivationFunctionType.Sqrt,
                )
                nc.scalar.activation(
                    out=dsl, in_=dsl,
                    func=mybir.ActivationFunctionType.Relu, bias=margneg,
                )
                hsl = hh[:, b0:b1]
                nc.vector.tensor_mul(hsl, dsl, dsl)
                wsl = hhw[:, b0:b1]
                nc.vector.tensor_mul(wsl, hsl, invc_all[:, b0:b1])
                nc.vector.tensor_reduce(
                    out=rsacc[:, qi : qi + 1], in_=wsl,
                    axis=mybir.AxisListType.X, op=mybir.AluOpType.add,
                )

        # ---------- finals: reduce per-quarter partials ----------
        rowsum = singles.tile([P, 1], F32)
        nc.vector.tensor_reduce(
            out=rowsum, in_=rsacc[:, : len(fin_oc)],
            axis=mybir.AxisListType.X, op=mybir.AluOpType.add,
        )
        ones128 = singles.tile([P, 1], F32)
        nc.gpsimd.memset(ones128, 1.0)
        psL = psS.tile([1, 1], F32, tag="small")
        nc.tensor.matmul(psL, rowsum, ones128)
        tot = wpool.tile([1, 3], F32, tag="tot")
        nc.scalar.copy(out=tot[:, 0:1], in_=psL)
        nc.scalar.copy(out=tot[:, 1:3], in_=ir)
        nc.sync.dma_start(out=out_d, in_=tot[0:1, :])

    nc.compile()
    return nc


_NC_CACHE = {}


def _get_program(tpc):
    if tpc not in _NC_CACHE:
        _NC_CACHE[tpc] = build_program(tpc)
    return _NC_CACHE[tpc]


def kernel(features, labels, num_clusters):
    features = np.asarray(features)
    labels = np.asarray(labels)
    n_total = features.shape[0]
    n_core = n_total // N_CORES
    tpc = math.ceil(n_core / P)
    nc = _get_program(tpc)
    in_maps = _host_prep(features, labels, tpc)
    res = run_bass_kernel_spmd(nc, in_maps, list(range(N_CORES)))
    intra_sum = sum(float(res.results[c]["out"][0]) for c in range(N_CORES))
    inter_sum = float(res.results[0]["out"][1])
    reg_sum = float(res.results[0]["out"][2])
    total = (
        intra_sum / K
        + inter_sum / (K * (K - 1))
        + 0.001 * reg_sum / K
    )
    return np.float32(total)


# revision 99
# speedup vs baseline: 1.0264x; 1.0073x over previous
# kernel.py — DiscriminativeLoss on 8 TRN2 NeuronCores (Bass/Tile, SPMD).
#
# Math (matches reference):
#   counts_k = #{i: l_i = k};  S_k = sum_{i in k} x_i;  mu_k = S_k / max(c_k, 1)
#   intra = (1/K) * sum_i invc_{l_i} * relu(||x_i - mu_{l_i} + eps|| - 1.5)^2
#   inter = sum_{a != b} relu(1 - ||(mu_a + eps) - mu_b||)^2 / (K*(K-1))
#   reg   = (1/K) * sum_k ||mu_k + eps||
#   total = intra + inter + 0.001 * reg
#
# V2 design (engine-balanced, cost-model driven):
#   pass 1: one-hot H2 [P, K, jn] via DVE/Pool tensor_tensor is_equal (2x
#     mode: all operands 2-byte packed SBUF); PE matmul lhsT=Xe [128,33],
#     rhs=H2[:, :, j] accumulates S^T = [S | counts] in PSUM [33, 64].
#   AllReduce [33, 64]; stats (mu, 1/c, sqrt(1/c), inter/reg losses) on
#     Act/Pool/PE only, keeping DVE free.
#   pass 2: paired transposed one-hot ht [128, 128] per tile-pair (A on
#     partitions 0-63, B on 64-127) built from broadcast-DMA'd labels via
#     DVE tensor_single_scalar is_equal (4x mode). Per tile, TWO accumulating
#     matmuls produce diff = x - (mu - eps) directly in PSUM:
#       psD  = ht_half^T @ [eps - mu | sqrt(1/c) - 1]   (gather, negated)
#       psD += I_128    @ Xe_tile                        ([x | 1])
#     -> psD = [x - mu + eps | sqrt(1/c)].
#   Act Square psD -> sq bf16 (col 32 squares to 1/c); DVE/Pool tensor_reduce
#     over D -> d2; DVE copies col 32 -> invc_all.
#   finals: dist=sqrt(d2), h=relu(dist-1.5) on Act; intra partial
#     sum_i h^2 * invc via DVE mults + tensor_tensor_reduce + PE ones-matmul.
import math
import numpy as np
from contextlib import ExitStack

import concourse.bass as bass
import concourse.bacc as bacc
import concourse.tile as tile
import concourse.mybir as mybir
from concourse.bass_utils import run_bass_kernel_spmd

F32 = mybir.dt.float32
BF16 = mybir.dt.bfloat16
I16 = mybir.dt.int16

N_CORES = 8
K = 64
D = 32
P = 128
EPS = 1e-8
PAD_LABEL = 999  # never matches any one-hot column

INTRA_MARGIN = 1.5
INTER_MARGIN2 = 1.0  # 2 * 0.5

J1 = 40      # pass-1 chunk width (tiles)
NACT = 0     # pass-1 tiles whose one-hot is built on Act (PE rank-3 + relu)
TPAIR = 28   # tile-pairs per ht chunk (4 gather groups of 7 pairs)
JMG = 14     # tiles per PSUM gather group (7 pairs)


def _host_prep(features, labels, tpc):
    """Shard + relayout on host. Returns per-core input dicts."""
    n_total = features.shape[0]
    n_core = n_total // N_CORES
    n_pad = P * tpc
    npair = (tpc + 1) // 2
    import ml_dtypes

    in_maps = []
    for c in range(N_CORES):
        f = np.asarray(features[c * n_core : (c + 1) * n_core], dtype=np.float32)
        l = np.asarray(labels[c * n_core : (c + 1) * n_core], dtype=np.int64)
        if n_pad > n_core:
            f = np.concatenate([f, np.zeros((n_pad - n_core, D), np.float32)], axis=0)
            l = np.concatenate([l, np.full((n_pad - n_core,), PAD_LABEL, np.int64)])
        # Xe: [P, tpc, 33] bf16, col 32 = 1/256 (exact in bf16; keeps the
        # sqrt(1/c) gather free of bf16 cancellation); point i = (i%P, i//P)
        xe = np.full((n_pad, D + 1), 1.0 / 256.0, np.float32)
        xe[:, :D] = f
        xe = xe.reshape(P, tpc, D + 1).astype(ml_dtypes.bfloat16)
        lpj = l.reshape(P, tpc)  # [point-in-tile, tile]
        l_pm = lpj.astype(np.int16)
        # paired label broadcast for ht, fully materialized on host:
        # [128, npair*128] int16, rows 0-63 = labels of tile 2jj, rows
        # 64-127 = labels of tile 2jj+1 (one contiguous DMA per chunk)
        ltm = lpj.T.astype(np.int16)  # [tpc, P]
        l_tma = np.full((npair, P), PAD_LABEL, np.int16)
        l_tmb = np.full((npair, P), PAD_LABEL, np.int16)
        l_tma[:] = ltm[0::2]
        nb = tpc // 2
        l_tmb[:nb] = ltm[1::2]
        l2full = np.empty((P, npair * P), np.int16)
        l2full[:K] = np.broadcast_to(
            l_tma.reshape(1, npair * P), (K, npair * P)
        )
        l2full[K:] = np.broadcast_to(
            l_tmb.reshape(1, npair * P), (K, npair * P)
        )
        # iotarep [P, K, J1] int16: value k at [:, k, :]
        iotarep = np.tile(
            np.arange(K, dtype=np.int16)[None, :, None], (P, 1, J1)
        )
        # rank-2 one-hot operands for the Act-built pass-1 tail (last NACT
        # tiles): per tail tile r, rows [1; l] live at partitions
        # 2*(r%64), +1, column band r//64 (PAD remapped to 100; all values
        # exact in bf16). PE gives (k - l) exactly; Act Square + Relu(1-x)
        # recover the one-hot.
        n_act_t = min(NACT, tpc)
        lsm = np.where(lpj == PAD_LABEL, 100, lpj).T.astype(np.int64)  # [tpc, P]
        nbands = max(1, math.ceil(n_act_t / 3))
        lr3h = np.zeros((P, nbands * P), np.float32)
        for r in range(n_act_t):
            s, b = r % 3, r // 3  # slot partition offsets 0/32/64 only
            lr3h[32 * s, b * P : (b + 1) * P] = 1.0
            lr3h[32 * s + 1, b * P : (b + 1) * P] = lsm[r]
        lr3h = lr3h.astype(ml_dtypes.bfloat16)
        kv = np.arange(K, dtype=np.float32)
        kvecrep = np.zeros((P, K), np.float32)
        for s in range(3):
            kvecrep[32 * s] = kv
            kvecrep[32 * s + 1] = -1.0
        kvecrep = kvecrep.astype(ml_dtypes.bfloat16)  # [128, K]
        in_maps.append(
            {
                "xe": np.ascontiguousarray(xe),
                "labels_pm": np.ascontiguousarray(l_pm),
                "l2full": l2full,
                "iotarep": np.ascontiguousarray(iotarep),
                "lr3h": np.ascontiguousarray(lr3h),
                "kvecrep": np.ascontiguousarray(kvecrep),
                "iotacol2": np.concatenate(
                    [np.arange(K), np.arange(K)]
                ).astype(np.float32).reshape(P, 1),
                "id128": np.eye(P, dtype=ml_dtypes.bfloat16),
                "idrep": np.ascontiguousarray(np.hstack(
                    [np.eye(K), np.eye(K)]).astype(ml_dtypes.bfloat16)),
                "id33": np.eye(D + 1, dtype=np.float32),
                "id64": np.eye(K, dtype=np.float32),
                "eyeneg": (1.0 - np.eye(K, dtype=np.float32)).astype(
                    ml_dtypes.bfloat16
                ),
            }
        )
    return in_maps


def build_program(tpc, dve_sq_every=6, f1_dve_every=0, ht_bufs=7, l2_bufs=3, mg_bufs=4):
    """Build the SPMD Bass program. tpc = tiles per core."""
    nc = bacc.Bacc(
        "TRN2", target_bir_lowering=False, debug=False, num_devices=N_CORES
    )
    core_ids = list(range(N_CORES))
    npair = (tpc + 1) // 2

    xe_d = nc.dram_tensor("xe", [P, tpc, D + 1], BF16, kind="ExternalInput").ap()
    lpm_d = nc.dram_tensor("labels_pm", [P, tpc], I16, kind="ExternalInput").ap()
    l2f_d = nc.dram_tensor("l2full", [P, npair * P], I16, kind="ExternalInput").ap()
    iotarep_d = nc.dram_tensor("iotarep", [P, K, J1], I16, kind="ExternalInput").ap()
    n_act_tiles = min(NACT, tpc)
    n_dve_tiles = tpc - n_act_tiles
    nbands = max(1, math.ceil(n_act_tiles / 3))
    lr3h_d = nc.dram_tensor("lr3h", [P, nbands * P], BF16, kind="ExternalInput").ap()
    kvecrep_d = nc.dram_tensor("kvecrep", [P, K], BF16, kind="ExternalInput").ap()
    iotacol2_d = nc.dram_tensor("iotacol2", [P, 1], F32, kind="ExternalInput").ap()
    id128_d = nc.dram_tensor("id128", [P, P], BF16, kind="ExternalInput").ap()
    idrep_d = nc.dram_tensor("idrep", [K, P], BF16, kind="ExternalInput").ap()
    id33_d = nc.dram_tensor("id33", [D + 1, D + 1], F32, kind="ExternalInput").ap()
    id64_d = nc.dram_tensor("id64", [K, K], F32, kind="ExternalInput").ap()
    eyeneg_d = nc.dram_tensor("eyeneg", [K, K], BF16, kind="ExternalInput").ap()
    out_d = nc.dram_tensor("out", [3], F32, kind="ExternalOutput").ap()

    n_chunks1 = math.ceil(tpc / J1)
    n_oc = math.ceil(npair / TPAIR)

    with tile.TileContext(nc, num_cores=N_CORES) as tc, ExitStack() as ctx:
        singles = ctx.enter_context(tc.tile_pool(name="singles", bufs=1))
        xpool = ctx.enter_context(tc.tile_pool(name="xpool", bufs=1))
        h2pool = ctx.enter_context(tc.tile_pool(name="h2pool", bufs=3))
        hqpool = ctx.enter_context(tc.tile_pool(name="hqpool", bufs=2))
        l2pool = ctx.enter_context(tc.tile_pool(name="l2pool", bufs=l2_bufs))
        htpool = ctx.enter_context(tc.tile_pool(name="htpool", bufs=ht_bufs))
        sqpool = ctx.enter_context(tc.tile_pool(name="sqpool", bufs=3))  # sq/f1..f4/cpy tags
        wpool = ctx.enter_context(tc.tile_pool(name="wpool", bufs=2))
        psA = ctx.enter_context(tc.tile_pool(name="psA", bufs=1, space="PSUM"))
        psQp = ctx.enter_context(tc.tile_pool(name="psQp", bufs=1, space="PSUM"))
        psMg = ctx.enter_context(tc.tile_pool(name="psMg", bufs=mg_bufs, space="PSUM"))
        psS = ctx.enter_context(tc.tile_pool(name="psS", bufs=2, space="PSUM"))
        dram = ctx.enter_context(tc.tile_pool(name="dram", bufs=2, space="DRAM"))

        # ---------- constants ----------
        lpm = singles.tile([P, tpc], I16)
        nc.sync.dma_start(out=lpm, in_=lpm_d)
        iotarep = singles.tile([P, K, J1], I16)
        nc.sync.dma_start(out=iotarep, in_=iotarep_d)
        iotacol2 = singles.tile([P, 1], F32)
        nc.sync.dma_start(out=iotacol2, in_=iotacol2_d)
        id128 = singles.tile([P, P], BF16)
        nc.sync.dma_start(out=id128, in_=id128_d)
        idrep = singles.tile([K, P], BF16)
        nc.sync.dma_start(out=idrep, in_=idrep_d)
        id33 = singles.tile([D + 1, D + 1], F32)
        nc.sync.dma_start(out=id33, in_=id33_d)
        id64 = singles.tile([K, K], F32)
        nc.sync.dma_start(out=id64, in_=id64_d)
        eyeneg = singles.tile([K, K], BF16)
        nc.sync.dma_start(out=eyeneg, in_=eyeneg_d)
        d2all = singles.tile([P, tpc], F32)
        invc_all = singles.tile([P, tpc], BF16)
        hh = singles.tile([P, tpc], F32)
        hhw = singles.tile([P, tpc], F32)
        rsacc = singles.tile([P, n_oc], F32)
        margneg = singles.tile([P, 1], F32)
        nc.gpsimd.memset(margneg, -float(INTRA_MARGIN))

        # l2 chunk DMA helper (host-materialized paired label broadcast)
        def issue_l2(oc):
            t0 = oc * TPAIR
            tn = min(TPAIR, npair - t0)
            l2 = l2pool.tile([P, TPAIR * P], I16, tag="l2")
            nc.sync.dma_start(
                out=l2[:, : tn * P], in_=l2f_d[:, t0 * P : (t0 + tn) * P]
            )
            return l2, tn

        # ---------- pass 1: segment sums ----------
        # xe DMAs issued first so pass-1 is never starved by the (large)
        # l2 broadcast transfers; l2 chunks are issued after so the
        # collective is not queued behind them on the DMA engines.
        # The LAST n_act_tiles tiles use an Act-engine one-hot instead of
        # DVE: PE rank-3 matmul gives (k - l)^2 in PSUM, Act relu(1 - x)
        # turns it into the one-hot (Act is otherwise idle before the
        # collective; this shortens the DVE-bound pass-1 phase).
        lr3 = singles.tile([P, nbands * P], BF16)
        nc.sync.dma_start(out=lr3, in_=lr3h_d)
        kvec = singles.tile([P, K], BF16)
        nc.sync.dma_start(out=kvec, in_=kvecrep_d)
        psumS = psA.tile([D + 1, K], F32)
        l2_tiles = []
        lc = 0
        t_done = 0
        # Act-built one-hot groups (7 tiles per PSUM bank), interleaved
        # among the DVE-built chunks so the PE queue never stalls long on
        # the PE->Act->PE round trip; Act is otherwise idle pre-collective.
        JQ = 7
        n_qgroups = math.ceil(n_act_tiles / JQ)

        def emit_act_group(qg):
            global_t = globals()  # noqa - placeholder
        def act_group(qg, t_done):
            q0 = qg * JQ
            qn = min(JQ, n_act_tiles - q0)
            nc.sync.dma_start(
                out=xe[:, q0 : q0 + qn, :], in_=xe_d[:, q0 : q0 + qn, :]
            )
            psQ = psQp.tile([P, JQ, K], F32, tag="psq")
            for t in range(qn):
                r = q0 + t
                s, b = r % 3, r // 3
                nc.tensor.matmul(
                    psQ[:, t, :],
                    lr3[32 * s : 32 * s + 2, b * P : (b + 1) * P],
                    kvec[32 * s : 32 * s + 2, :],
                    start=True, stop=True,
                )
            h2sq = hqpool.tile([P, JQ, K], BF16, tag="h2sq")
            nc.scalar.activation(
                out=h2sq[:, :qn, :], in_=psQ[:, :qn, :],
                func=mybir.ActivationFunctionType.Square,
            )
            h2a = hqpool.tile([P, JQ, K], BF16, tag="h2a")
            nc.scalar.activation(
                out=h2a[:, :qn, :], in_=h2sq[:, :qn, :],
                func=mybir.ActivationFunctionType.Relu, bias=1.0, scale=-1.0,
            )
            for t in range(qn):
                nc.tensor.matmul(
                    psumS,
                    xe[:, q0 + t, :],
                    h2a[:, t, :],
                    start=(t_done == 0),
                    stop=(t_done == tpc - 1),
                )
                t_done += 1
            return t_done

        # DVE-built chunks; xe is streamed through a rolling pool (the
        # full-size resident copy is gone — pass 2 re-streams its own xe
        # chunks during the otherwise idle collective window, freeing
        # ~64KB of SBUF for a much deeper ht pool)
        n_chunks1d = math.ceil(n_dve_tiles / J1)
        qg_next = 0
        xe = xpool.tile([P, tpc, D + 1], BF16)
        for c in range(n_chunks1d):
            j0 = n_act_tiles + c * J1
            jn = min(J1, tpc - j0)
            nc.sync.dma_start(
                out=xe[:, j0 : j0 + jn, :], in_=xe_d[:, j0 : j0 + jn, :]
            )
            h2 = h2pool.tile([P, K, J1], BF16, tag="h2")
            nc.vector.tensor_tensor(
                h2[:, :, :jn],
                lpm[:, None, j0 : j0 + jn].to_broadcast((P, K, jn)),
                iotarep[:, :, :jn],
                mybir.AluOpType.is_equal,
            )
            for j in range(jn):
                nc.tensor.matmul(
                    psumS,
                    xe[:, j0 + j, :],
                    h2[:, :, j],
                    start=(t_done == 0),
                    stop=(t_done == tpc - 1),
                )
                t_done += 1
        # l2 label chunks stream right behind xe on the DMA engines
        while lc < n_oc:
            l2_tiles.append(issue_l2(lc))
            lc += 1
        # ---------- AllGather segment sums + local reduce ----------
        # (AllGather avoids the cost model's 1.875x AllReduce penalty; the
        #  8-way sum is 3 cheap tree adds done locally)
        sg_local = wpool.tile([D + 1, K], BF16, tag="sg")
        nc.scalar.copy(out=sg_local, in_=psumS)
        cc_in = dram.tile([D + 1, K], BF16)
        cc_out = dram.tile([N_CORES, D + 1, K], BF16)
        nc.gpsimd.dma_start(out=cc_in, in_=sg_local)
        nc.gpsimd.collective_compute(
            "AllGather",
            mybir.AluOpType.bypass,
            replica_groups=[core_ids],
            ins=[cc_in.opt()],
            outs=[cc_out.opt()],
        )
        sg8 = wpool.tile([D + 1, N_CORES, K], BF16, tag="sg8")
        ccf = cc_out[0, 0, 0]  # base AP for offset/tensor
        nc.gpsimd.dma_start(
            out=sg8,
            in_=bass.AP(
                tensor=ccf.tensor, offset=ccf.offset,
                ap=[[K, D + 1], [(D + 1) * K, N_CORES], [1, K]],
            ),
        )
        with nc.allow_low_precision(reason="bf16 cross-core segment sums"):
            sg4 = wpool.tile([D + 1, 4, K], BF16, tag="sg4")
            nc.vector.tensor_add(sg4, sg8[:, :4, :], sg8[:, 4:, :])
            sg2t = wpool.tile([D + 1, 2, K], BF16, tag="sg2t")
            nc.vector.tensor_add(sg2t, sg4[:, :2, :], sg4[:, 2:, :])
        sg = wpool.tile([D + 1, K], F32, tag="sg2")
        nc.vector.tensor_tensor(
            sg, sg2t[:, 0, :], sg2t[:, 1, :], mybir.AluOpType.add
        )

        # ---------- ht builds (no AR dependency) ----------
        ht_tiles = []
        for oc in range(n_oc):
            l2, tn = l2_tiles[oc]
            ht = htpool.tile([P, TPAIR * P], BF16, tag="ht")
            nc.vector.tensor_single_scalar(
                ht[:, : tn * P], l2[:, : tn * P], iotacol2,
                mybir.AluOpType.is_equal,
            )
            ht_tiles.append(ht)

        # ---------- stats (Act/Pool/PE only; DVE stays on one-hot work) ----
        psW = psS.tile([K, D + 1], F32, tag="small")
        nc.tensor.transpose(psW, sg, id33)
        W = wpool.tile([K, D + 1], F32, tag="w")  # [S_k | c_k]
        nc.scalar.copy(out=W, in_=psW)
        safec = wpool.tile([K, 1], F32, tag="safec")
        nc.gpsimd.tensor_scalar(
            safec, W[:, D : D + 1], 256.0, 1.0,
            mybir.AluOpType.mult, mybir.AluOpType.max,
        )
        invc = wpool.tile([K, 1], F32, tag="invc")
        nc.vector.reciprocal(invc, safec)
        svp = wpool.tile([K, 1], F32, tag="svp")  # sqrt(1/c)
        nc.scalar.activation(
            out=svp, in_=invc, func=mybir.ActivationFunctionType.Sqrt
        )
        mu = wpool.tile([K, D], F32, tag="mu")
        nc.gpsimd.tensor_mul(mu, W[:, :D], invc.to_broadcast((K, D)))
        # table2 [128, 33] bf16 = [eps - mu | sqrt(1/c) - 1], rows replicated
        table2 = singles.tile([P, D + 1], BF16)
        nc.scalar.activation(
            out=table2[:K, :D], in_=mu,
            func=mybir.ActivationFunctionType.Copy, bias=EPS, scale=-1.0,
        )
        nc.scalar.activation(
            out=table2[:K, D : D + 1], in_=svp,
            func=mybir.ActivationFunctionType.Copy, bias=-1.0 / 256.0,
        )
        psTF = psS.tile([P, D + 1], F32, tag="small")
        nc.tensor.matmul(psTF, idrep, table2[:K, :])
        table2f = singles.tile([P, D + 1], BF16)
        nc.scalar.copy(out=table2f, in_=psTF)

        # ---------- inter + reg losses (Act/Pool/PE) ----------
        mup = wpool.tile([K, D], F32, tag="mup")  # mu + eps
        nc.scalar.activation(
            out=mup, in_=mu, func=mybir.ActivationFunctionType.Copy, bias=EPS
        )
        qsc = wpool.tile([K, D], F32, tag="qsc")
        nc.gpsimd.tensor_mul(qsc, mu, mu)
        q = wpool.tile([K, 1], F32, tag="q")  # ||mu||^2
        nc.vector.tensor_reduce(
            out=q, in_=qsc, axis=mybir.AxisListType.X, op=mybir.AluOpType.add
        )
        qpsc = wpool.tile([K, D], F32, tag="qpsc")
        nc.gpsimd.tensor_mul(qpsc, mup, mup)
        qp = wpool.tile([K, 1], F32, tag="qp")  # ||mu + eps||^2
        nc.vector.tensor_reduce(
            out=qp, in_=qpsc, axis=mybir.AxisListType.X, op=mybir.AluOpType.add
        )
        # pd2[a,b] = qp_a - 2*mup_a.mu_b + q_b via [ -2*mup | qp | 1 ] x [ mu | 1 | q ]
        ab = wpool.tile([K, D + 2], F32, tag="ab")
        nc.scalar.mul(out=ab[:, :D], in_=mup, mul=-2.0)
        nc.scalar.copy(out=ab[:, D : D + 1], in_=qp)
        nc.gpsimd.memset(ab[:, D + 1 : D + 2], 1.0)
        bb = wpool.tile([K, D + 2], F32, tag="bb")
        nc.scalar.copy(out=bb[:, :D], in_=mu)
        nc.gpsimd.memset(bb[:, D : D + 1], 1.0)
        nc.scalar.copy(out=bb[:, D + 1 : D + 2], in_=q)
        psT = psS.tile([D + 2, K], F32, tag="small")
        nc.tensor.transpose(psT, ab, id64)
        atp = wpool.tile([D + 2, K], F32, tag="atp")
        nc.scalar.copy(out=atp, in_=psT)
        psT2 = psS.tile([D + 2, K], F32, tag="small")
        nc.tensor.transpose(psT2, bb, id64)
        btp = wpool.tile([D + 2, K], F32, tag="btp")
        nc.scalar.copy(out=btp, in_=psT2)
        psPD = psS.tile([K, K], F32, tag="small")
        nc.tensor.matmul(psPD, atp, btp)
        pdc = wpool.tile([K, K], F32, tag="pdc")
        nc.vector.tensor_scalar_max(pdc, psPD, 0.0)
        pdist = wpool.tile([K, K], F32, tag="pdist")
        nc.scalar.activation(
            out=pdist, in_=pdc, func=mybir.ActivationFunctionType.Sqrt
        )
        hingeI = wpool.tile([K, K], F32, tag="hingeI")
        nc.scalar.activation(
            out=hingeI, in_=pdist, func=mybir.ActivationFunctionType.Relu,
            bias=float(INTER_MARGIN2), scale=-1.0,
        )
        hm = wpool.tile([K, K], F32, tag="hm")
        nc.gpsimd.tensor_mul(hm, hingeI, eyeneg)
        hm2 = wpool.tile([K, K], F32, tag="hm2")
        nc.gpsimd.tensor_mul(hm2, hm, hm)
        interp = wpool.tile([K, 1], F32, tag="interp")
        nc.vector.tensor_reduce(
            out=interp, in_=hm2, axis=mybir.AxisListType.X,
            op=mybir.AluOpType.add,
        )
        sqp = wpool.tile([K, 1], F32, tag="sqp")  # ||mu + eps||
        nc.scalar.activation(
            out=sqp, in_=qp, func=mybir.ActivationFunctionType.Sqrt
        )
        cat2 = wpool.tile([K, 2], F32, tag="cat2")
        nc.scalar.copy(out=cat2[:, 0:1], in_=interp)
        nc.scalar.copy(out=cat2[:, 1:2], in_=sqp)
        ones64 = singles.tile([K, 1], F32)
        nc.gpsimd.memset(ones64, 1.0)
        psIR = psS.tile([1, 2], F32, tag="small")
        nc.tensor.matmul(psIR, ones64, cat2)
        ir = wpool.tile([1, 2], F32, tag="ir")  # [inter_sum, reg_sum]
        nc.scalar.copy(out=ir, in_=psIR)

        # ---------- pass 2: gather + diff in PSUM, square, fold-reduce ------
        fin_oc = sorted(set(
            [n_oc - 1] + [max(0, (n_oc * (q + 1)) // 4 - 1) for q in range(3)]
        ))
        fin_base = []
        prev = 0
        for oc_ in fin_oc:
            fin_base.append(prev)
            pc_ = min(TPAIR, npair - oc_ * TPAIR)
            prev = min(oc_ * TPAIR * 2 + pc_ * 2, tpc)
        for oc in range(n_oc):
            ht = ht_tiles[oc]
            pc = min(TPAIR, npair - oc * TPAIR)
            cbase = oc * TPAIR * 2        # first global tile of this chunk
            ctn = min(pc * 2, tpc - cbase)  # tiles in this chunk
            # one sq tile per ht chunk (up to 56 tiles), 4 PSUM groups
            sq = sqpool.tile([P, TPAIR * 2, D + 1], BF16, tag="sq")
            for g in range(math.ceil(pc / (JMG // 2))):
                p0 = g * (JMG // 2)
                pn = min(JMG // 2, pc - p0)
                jbase = (oc * TPAIR + p0) * 2  # first global tile of group
                nt = min(pn * 2, tpc - jbase)
                psD = psMg.tile([P, JMG, D + 1], F32, tag="psd")
                for lp in range(pn):
                    for half in range(2):
                        t = lp * 2 + half
                        if t >= nt:
                            break
                        colp = p0 + lp
                        nc.tensor.matmul(
                            psD[:, t, :],
                            ht[64 * half : 64 * (half + 1),
                               colp * P : (colp + 1) * P],
                            table2f[64 * half : 64 * (half + 1), :],
                            start=True, stop=False,
                        )
                        nc.tensor.matmul(
                            psD[:, t, :], id128,
                            xe[:, cbase + p0 * 2 + t, :],
                            start=False, stop=True,
                        )
                t0 = p0 * 2
                if dve_sq_every and (oc * 4 + g) % dve_sq_every == dve_sq_every - 1:
                    # DVE square: PSUM copy then bf16 self-mult (one PSUM
                    # input per instruction as required by hardware)
                    cpy = sqpool.tile([P, JMG, D + 1], BF16, tag="cpy")
                    nc.vector.tensor_scalar_add(
                        cpy[:, :nt, :], psD[:, :nt, :], 0.0
                    )
                    nc.vector.tensor_mul(
                        sq[:, t0 : t0 + nt, :], cpy[:, :nt, :], cpy[:, :nt, :]
                    )
                else:
                    nc.scalar.activation(
                        out=sq[:, t0 : t0 + nt, :], in_=psD[:, :nt, :],
                        func=mybir.ActivationFunctionType.Square,
                    )
            # bf16 fold-tree reduce over D (2x DVE mode); f1 of every other
            # chunk goes to Pool to offload DVE
            with nc.allow_low_precision(reason="bf16 partial sums of d2"):
                f1 = sqpool.tile([P, TPAIR * 2, 16], BF16, tag="f1")
                f1eng = nc.vector if (f1_dve_every and oc % f1_dve_every == f1_dve_every - 1) else nc.gpsimd
                f1eng.tensor_add(
                    f1[:, :ctn, :], sq[:, :ctn, 0:16], sq[:, :ctn, 16:32]
                )
                f2 = sqpool.tile([P, TPAIR * 2, 8], BF16, tag="f2")
                nc.vector.tensor_add(
                    f2[:, :ctn, :], f1[:, :ctn, 0:8], f1[:, :ctn, 8:16]
                )
                f3 = sqpool.tile([P, TPAIR * 2, 4], BF16, tag="f3")
                nc.vector.tensor_add(
                    f3[:, :ctn, :], f2[:, :ctn, 0:4], f2[:, :ctn, 4:8]
                )
                f4 = sqpool.tile([P, TPAIR * 2, 2], BF16, tag="f4")
                nc.vector.tensor_add(
                    f4[:, :ctn, :], f3[:, :ctn, 0:2], f3[:, :ctn, 2:4]
                )
            nc.vector.tensor_tensor(
                d2all[:, cbase : cbase + ctn],
                f4[:, :ctn, 0], f4[:, :ctn, 1], mybir.AluOpType.add,
            )
            nc.gpsimd.tensor_scalar_add(
                invc_all[:, cbase : cbase + ctn], sq[:, :ctn, D], 0.0
            )
            # quarter-granularity finals (keeps the serial tail short
            # without flooding Act with per-chunk overhead):
            # dist = sqrt(d2); h = relu(dist - 1.5); acc_q = sum h^2 * invc
            if oc in fin_oc:
                qi = fin_oc.index(oc)
                b0 = fin_base[qi]
                b1 = cbase + ctn
                dsl = d2all[:, b0:b1]
                nc.scalar.activation(
                    out=dsl, in_=dsl,
                    func=mybir.ActivationFunctionType.Sqrt,
                )
                nc.scalar.activation(
                    out=dsl, in_=dsl,
                    func=mybir.ActivationFunctionType.Relu, bias=margneg,
                )
                hsl = hh[:, b0:b1]
                nc.vector.tensor_mul(hsl, dsl, dsl)
                wsl = hhw[:, b0:b1]
                nc.vector.tensor_mul(wsl, hsl, invc_all[:, b0:b1])
                nc.vector.tensor_reduce(
                    out=rsacc[:, qi : qi + 1], in_=wsl,
                    axis=mybir.AxisListType.X, op=mybir.AluOpType.add,
                )

        # ---------- finals: reduce per-quarter partials ----------
        rowsum = singles.tile([P, 1], F32)
        nc.vector.tensor_reduce(
            out=rowsum, in_=rsacc[:, : len(fin_oc)],
            axis=mybir.AxisListType.X, op=mybir.AluOpType.add,
        )
        ones128 = singles.tile([P, 1], F32)
        nc.gpsimd.memset(ones128, 1.0)
        psL = psS.tile([1, 1], F32, tag="small")
        nc.tensor.matmul(psL, rowsum, ones128)
        tot = wpool.tile([1, 3], F32, tag="tot")
        nc.scalar.copy(out=tot[:, 0:1], in_=psL)
        nc.scalar.copy(out=tot[:, 1:3], in_=ir)
        nc.sync.dma_start(out=out_d, in_=tot[0:1, :])

    nc.compile()
    return nc


_NC_CACHE = {}


def _get_program(tpc):
    if tpc not in _NC_CACHE:
        _NC_CACHE[tpc] = build_program(tpc)
    return _NC_CACHE[tpc]


def kernel(features, labels, num_clusters):
    features = np.asarray(features)
    labels = np.asarray(labels)
    n_total = features.shape[0]
    n_core = n_total // N_CORES
    tpc = math.ceil(n_core / P)
    nc = _get_program(tpc)
    in_maps = _host_prep(features, labels, tpc)
    res = run_bass_kernel_spmd(nc, in_maps, list(range(N_CORES)))
    intra_sum = sum(float(res.results[c]["out"][0]) for c in range(N_CORES))
    inter_sum = float(res.results[0]["out"][1])
    reg_sum = float(res.results[0]["out"][2])
    total = (
        intra_sum / K
        + inter_sum / (K * (K - 1))
        + 0.001 * reg_sum / K
    )
    return np.float32(total)


# revision 100
# speedup vs baseline: 1.0553x; 1.0281x over previous
# kernel.py — DiscriminativeLoss on 8 TRN2 NeuronCores (Bass/Tile, SPMD).
#
# Math (matches reference):
#   counts_k = #{i: l_i = k};  S_k = sum_{i in k} x_i;  mu_k = S_k / max(c_k, 1)
#   intra = (1/K) * sum_i invc_{l_i} * relu(||x_i - mu_{l_i} + eps|| - 1.5)^2
#   inter = sum_{a != b} relu(1 - ||(mu_a + eps) - mu_b||)^2 / (K*(K-1))
#   reg   = (1/K) * sum_k ||mu_k + eps||
#   total = intra + inter + 0.001 * reg
#
# V2 design (engine-balanced, cost-model driven):
#   pass 1: one-hot H2 [P, K, jn] via DVE/Pool tensor_tensor is_equal (2x
#     mode: all operands 2-byte packed SBUF); PE matmul lhsT=Xe [128,33],
#     rhs=H2[:, :, j] accumulates S^T = [S | counts] in PSUM [33, 64].
#   AllReduce [33, 64]; stats (mu, 1/c, sqrt(1/c), inter/reg losses) on
#     Act/Pool/PE only, keeping DVE free.
#   pass 2: paired transposed one-hot ht [128, 128] per tile-pair (A on
#     partitions 0-63, B on 64-127) built from broadcast-DMA'd labels via
#     DVE tensor_single_scalar is_equal (4x mode). Per tile, TWO accumulating
#     matmuls produce diff = x - (mu - eps) directly in PSUM:
#       psD  = ht_half^T @ [eps - mu | sqrt(1/c) - 1]   (gather, negated)
#       psD += I_128    @ Xe_tile                        ([x | 1])
#     -> psD = [x - mu + eps | sqrt(1/c)].
#   Act Square psD -> sq bf16 (col 32 squares to 1/c); DVE/Pool tensor_reduce
#     over D -> d2; DVE copies col 32 -> invc_all.
#   finals: dist=sqrt(d2), h=relu(dist-1.5) on Act; intra partial
#     sum_i h^2 * invc via DVE mults + tensor_tensor_reduce + PE ones-matmul.
import math
import numpy as np
from contextlib import ExitStack

import concourse.bass as bass
import concourse.bacc as bacc
import concourse.tile as tile
import concourse.mybir as mybir
from concourse.bass_utils import run_bass_kernel_spmd

F32 = mybir.dt.float32
BF16 = mybir.dt.bfloat16
I16 = mybir.dt.int16

N_CORES = 8
K = 64
D = 32
P = 128
EPS = 1e-8
PAD_LABEL = 999  # never matches any one-hot column

INTRA_MARGIN = 1.5
INTER_MARGIN2 = 1.0  # 2 * 0.5

J1 = 40      # pass-1 chunk width (tiles)
NACT = 0     # pass-1 tiles whose one-hot is built on Act (PE rank-3 + relu)
TPAIR = 28   # tile-pairs per ht chunk (4 gather groups of 7 pairs)
JMG = 14     # tiles per PSUM gather group (7 pairs)


def _host_prep(features, labels, tpc):
    """Shard + relayout on host. Returns per-core input dicts."""
    n_total = features.shape[0]
    n_core = n_total // N_CORES
    n_pad = P * tpc
    npair = (tpc + 1) // 2
    import ml_dtypes

    in_maps = []
    for c in range(N_CORES):
        f = np.asarray(features[c * n_core : (c + 1) * n_core], dtype=np.float32)
        l = np.asarray(labels[c * n_core : (c + 1) * n_core], dtype=np.int64)
        if n_pad > n_core:
            f = np.concatenate([f, np.zeros((n_pad - n_core, D), np.float32)], axis=0)
            l = np.concatenate([l, np.full((n_pad - n_core,), PAD_LABEL, np.int64)])
        # Xe: [P, tpc, 33] bf16, col 32 = 1/256 (exact in bf16; keeps the
        # sqrt(1/c) gather free of bf16 cancellation); point i = (i%P, i//P)
        xe = np.full((n_pad, D + 1), 1.0 / 256.0, np.float32)
        xe[:, :D] = f
        xe = xe.reshape(P, tpc, D + 1).astype(ml_dtypes.bfloat16)
        lpj = l.reshape(P, tpc)  # [point-in-tile, tile]
        l_pm = lpj.astype(np.int16)
        # paired label broadcast for ht, fully materialized on host:
        # [128, npair*128] int16, rows 0-63 = labels of tile 2jj, rows
        # 64-127 = labels of tile 2jj+1 (one contiguous DMA per chunk)
        ltm = lpj.T.astype(np.int16)  # [tpc, P]
        l_tma = np.full((npair, P), PAD_LABEL, np.int16)
        l_tmb = np.full((npair, P), PAD_LABEL, np.int16)
        l_tma[:] = ltm[0::2]
        nb = tpc // 2
        l_tmb[:nb] = ltm[1::2]
        l2full = np.empty((P, npair * P), np.int16)
        l2full[:K] = np.broadcast_to(
            l_tma.reshape(1, npair * P), (K, npair * P)
        )
        l2full[K:] = np.broadcast_to(
            l_tmb.reshape(1, npair * P), (K, npair * P)
        )
        # iotarep [P, K, J1] int16: value k at [:, k, :]
        iotarep = np.tile(
            np.arange(K, dtype=np.int16)[None, :, None], (P, 1, J1)
        )
        # rank-2 one-hot operands for the Act-built pass-1 tail (last NACT
        # tiles): per tail tile r, rows [1; l] live at partitions
        # 2*(r%64), +1, column band r//64 (PAD remapped to 100; all values
        # exact in bf16). PE gives (k - l) exactly; Act Square + Relu(1-x)
        # recover the one-hot.
        n_act_t = min(NACT, tpc)
        lsm = np.where(lpj == PAD_LABEL, 100, lpj).T.astype(np.int64)  # [tpc, P]
        nbands = max(1, math.ceil(n_act_t / 3))
        lr3h = np.zeros((P, nbands * P), np.float32)
        for r in range(n_act_t):
            s, b = r % 3, r // 3  # slot partition offsets 0/32/64 only
            lr3h[32 * s, b * P : (b + 1) * P] = 1.0
            lr3h[32 * s + 1, b * P : (b + 1) * P] = lsm[r]
        lr3h = lr3h.astype(ml_dtypes.bfloat16)
        kv = np.arange(K, dtype=np.float32)
        kvecrep = np.zeros((P, K), np.float32)
        for s in range(3):
            kvecrep[32 * s] = kv
            kvecrep[32 * s + 1] = -1.0
        kvecrep = kvecrep.astype(ml_dtypes.bfloat16)  # [128, K]
        in_maps.append(
            {
                "xe": np.ascontiguousarray(xe),
                "labels_pm": np.ascontiguousarray(l_pm),
                "l2full": l2full,
                "iotarep": np.ascontiguousarray(iotarep),
                "lr3h": np.ascontiguousarray(lr3h),
                "kvecrep": np.ascontiguousarray(kvecrep),
                "iotacol2": np.concatenate(
                    [np.arange(K), np.arange(K)]
                ).astype(np.float32).reshape(P, 1),
                "id128": np.eye(P, dtype=ml_dtypes.bfloat16),
                "idrep": np.ascontiguousarray(np.hstack(
                    [np.eye(K), np.eye(K)]).astype(ml_dtypes.bfloat16)),
                "id33": np.eye(D + 1, dtype=np.float32),
                "id64": np.eye(K, dtype=np.float32),
                "eyeneg": (1.0 - np.eye(K, dtype=np.float32)).astype(
                    ml_dtypes.bfloat16
                ),
            }
        )
    return in_maps


def build_program(tpc, dve_sq_every=6, f1_dve_every=0, ht_bufs=7, l2_bufs=3, mg_bufs=4):
    """Build the SPMD Bass program. tpc = tiles per core."""
    nc = bacc.Bacc(
        "TRN2", target_bir_lowering=False, debug=False, num_devices=N_CORES
    )
    core_ids = list(range(N_CORES))
    npair = (tpc + 1) // 2

    xe_d = nc.dram_tensor("xe", [P, tpc, D + 1], BF16, kind="ExternalInput").ap()
    lpm_d = nc.dram_tensor("labels_pm", [P, tpc], I16, kind="ExternalInput").ap()
    l2f_d = nc.dram_tensor("l2full", [P, npair * P], I16, kind="ExternalInput").ap()
    iotarep_d = nc.dram_tensor("iotarep", [P, K, J1], I16, kind="ExternalInput").ap()
    n_act_tiles = min(NACT, tpc)
    n_dve_tiles = tpc - n_act_tiles
    nbands = max(1, math.ceil(n_act_tiles / 3))
    lr3h_d = nc.dram_tensor("lr3h", [P, nbands * P], BF16, kind="ExternalInput").ap()
    kvecrep_d = nc.dram_tensor("kvecrep", [P, K], BF16, kind="ExternalInput").ap()
    iotacol2_d = nc.dram_tensor("iotacol2", [P, 1], F32, kind="ExternalInput").ap()
    id128_d = nc.dram_tensor("id128", [P, P], BF16, kind="ExternalInput").ap()
    idrep_d = nc.dram_tensor("idrep", [K, P], BF16, kind="ExternalInput").ap()
    id33_d = nc.dram_tensor("id33", [D + 1, D + 1], F32, kind="ExternalInput").ap()
    id64_d = nc.dram_tensor("id64", [K, K], F32, kind="ExternalInput").ap()
    eyeneg_d = nc.dram_tensor("eyeneg", [K, K], BF16, kind="ExternalInput").ap()
    out_d = nc.dram_tensor("out", [3], F32, kind="ExternalOutput").ap()

    n_chunks1 = math.ceil(tpc / J1)
    n_oc = math.ceil(npair / TPAIR)

    with tile.TileContext(nc, num_cores=N_CORES) as tc, ExitStack() as ctx:
        singles = ctx.enter_context(tc.tile_pool(name="singles", bufs=1))
        xpool = ctx.enter_context(tc.tile_pool(name="xpool", bufs=1))
        h2pool = ctx.enter_context(tc.tile_pool(name="h2pool", bufs=3))
        hqpool = ctx.enter_context(tc.tile_pool(name="hqpool", bufs=2))
        l2pool = ctx.enter_context(tc.tile_pool(name="l2pool", bufs=l2_bufs))
        htpool = ctx.enter_context(tc.tile_pool(name="htpool", bufs=ht_bufs))
        sqpool = ctx.enter_context(tc.tile_pool(name="sqpool", bufs=3))  # sq/f1..f4/cpy tags
        wpool = ctx.enter_context(tc.tile_pool(name="wpool", bufs=2))
        psA = ctx.enter_context(tc.tile_pool(name="psA", bufs=1, space="PSUM"))
        psQp = ctx.enter_context(tc.tile_pool(name="psQp", bufs=1, space="PSUM"))
        psMg = ctx.enter_context(tc.tile_pool(name="psMg", bufs=mg_bufs, space="PSUM"))
        psS = ctx.enter_context(tc.tile_pool(name="psS", bufs=2, space="PSUM"))
        dram = ctx.enter_context(tc.tile_pool(name="dram", bufs=2, space="DRAM"))

        # ---------- constants ----------
        lpm = singles.tile([P, tpc], I16)
        nc.sync.dma_start(out=lpm, in_=lpm_d)
        iotarep = singles.tile([P, K, J1], I16)
        nc.sync.dma_start(out=iotarep, in_=iotarep_d)
        d2all = singles.tile([P, tpc], F32)
        invc_all = singles.tile([P, tpc], BF16)
        hh = singles.tile([P, tpc], F32)
        hhw = singles.tile([P, tpc], F32)
        rsacc = singles.tile([P, n_oc], F32)
        margneg = singles.tile([P, 1], F32)
        nc.gpsimd.memset(margneg, -float(INTRA_MARGIN))

        # l2 chunk DMA helper (host-materialized paired label broadcast)
        def issue_l2(oc):
            t0 = oc * TPAIR
            tn = min(TPAIR, npair - t0)
            l2 = l2pool.tile([P, TPAIR * P], I16, tag="l2")
            nc.sync.dma_start(
                out=l2[:, : tn * P], in_=l2f_d[:, t0 * P : (t0 + tn) * P]
            )
            return l2, tn

        # ---------- pass 1: segment sums ----------
        # xe DMAs issued first so pass-1 is never starved by the (large)
        # l2 broadcast transfers; l2 chunks are issued after so the
        # collective is not queued behind them on the DMA engines.
        # The LAST n_act_tiles tiles use an Act-engine one-hot instead of
        # DVE: PE rank-3 matmul gives (k - l)^2 in PSUM, Act relu(1 - x)
        # turns it into the one-hot (Act is otherwise idle before the
        # collective; this shortens the DVE-bound pass-1 phase).
        lr3 = singles.tile([P, nbands * P], BF16)
        nc.sync.dma_start(out=lr3, in_=lr3h_d)
        kvec = singles.tile([P, K], BF16)
        nc.sync.dma_start(out=kvec, in_=kvecrep_d)
        psumS = psA.tile([D + 1, K], F32)
        l2_tiles = []
        lc = 0
        t_done = 0
        # Act-built one-hot groups (7 tiles per PSUM bank), interleaved
        # among the DVE-built chunks so the PE queue never stalls long on
        # the PE->Act->PE round trip; Act is otherwise idle pre-collective.
        JQ = 7
        n_qgroups = math.ceil(n_act_tiles / JQ)

        def emit_act_group(qg):
            global_t = globals()  # noqa - placeholder
        def act_group(qg, t_done):
            q0 = qg * JQ
            qn = min(JQ, n_act_tiles - q0)
            nc.sync.dma_start(
                out=xe[:, q0 : q0 + qn, :], in_=xe_d[:, q0 : q0 + qn, :]
            )
            psQ = psQp.tile([P, JQ, K], F32, tag="psq")
            for t in range(qn):
                r = q0 + t
                s, b = r % 3, r // 3
                nc.tensor.matmul(
                    psQ[:, t, :],
                    lr3[32 * s : 32 * s + 2, b * P : (b + 1) * P],
                    kvec[32 * s : 32 * s + 2, :],
                    start=True, stop=True,
                )
            h2sq = hqpool.tile([P, JQ, K], BF16, tag="h2sq")
            nc.scalar.activation(
                out=h2sq[:, :qn, :], in_=psQ[:, :qn, :],
                func=mybir.ActivationFunctionType.Square,
            )
            h2a = hqpool.tile([P, JQ, K], BF16, tag="h2a")
            nc.scalar.activation(
                out=h2a[:, :qn, :], in_=h2sq[:, :qn, :],
                func=mybir.ActivationFunctionType.Relu, bias=1.0, scale=-1.0,
            )
            for t in range(qn):
                nc.tensor.matmul(
                    psumS,
                    xe[:, q0 + t, :],
                    h2a[:, t, :],
                    start=(t_done == 0),
                    stop=(t_done == tpc - 1),
                )
                t_done += 1
            return t_done

        # DVE-built chunks; xe is streamed through a rolling pool (the
        # full-size resident copy is gone — pass 2 re-streams its own xe
        # chunks during the otherwise idle collective window, freeing
        # ~64KB of SBUF for a much deeper ht pool)
        n_chunks1d = math.ceil(n_dve_tiles / J1)
        qg_next = 0
        xe = xpool.tile([P, tpc, D + 1], BF16)
        for c in range(n_chunks1d):
            j0 = n_act_tiles + c * J1
            jn = min(J1, tpc - j0)
            nc.sync.dma_start(
                out=xe[:, j0 : j0 + jn, :], in_=xe_d[:, j0 : j0 + jn, :]
            )
            h2 = h2pool.tile([P, K, J1], BF16, tag="h2")
            nc.vector.tensor_tensor(
                h2[:, :, :jn],
                lpm[:, None, j0 : j0 + jn].to_broadcast((P, K, jn)),
                iotarep[:, :, :jn],
                mybir.AluOpType.is_equal,
            )
            for j in range(jn):
                nc.tensor.matmul(
                    psumS,
                    xe[:, j0 + j, :],
                    h2[:, :, j],
                    start=(t_done == 0),
                    stop=(t_done == tpc - 1),
                )
                t_done += 1
        # stats / pass-2 constants (not needed until after the collective;
        # issuing them here keeps the first xe chunk off the startup path)
        iotacol2 = singles.tile([P, 1], F32)
        nc.sync.dma_start(out=iotacol2, in_=iotacol2_d)
        id128 = singles.tile([P, P], BF16)
        nc.sync.dma_start(out=id128, in_=id128_d)
        idrep = singles.tile([K, P], BF16)
        nc.sync.dma_start(out=idrep, in_=idrep_d)
        id33 = singles.tile([D + 1, D + 1], F32)
        nc.sync.dma_start(out=id33, in_=id33_d)
        id64 = singles.tile([K, K], F32)
        nc.sync.dma_start(out=id64, in_=id64_d)
        eyeneg = singles.tile([K, K], BF16)
        nc.sync.dma_start(out=eyeneg, in_=eyeneg_d)
        # l2 label chunks stream right behind xe on the DMA engines
        while lc < n_oc:
            l2_tiles.append(issue_l2(lc))
            lc += 1
        # ---------- AllGather segment sums + local reduce ----------
        # (AllGather avoids the cost model's 1.875x AllReduce penalty; the
        #  8-way sum is 3 cheap tree adds done locally)
        sg_local = wpool.tile([D + 1, K], BF16, tag="sg")
        nc.scalar.copy(out=sg_local, in_=psumS)
        cc_in = dram.tile([D + 1, K], BF16)
        cc_out = dram.tile([N_CORES, D + 1, K], BF16)
        nc.gpsimd.dma_start(out=cc_in, in_=sg_local)
        nc.gpsimd.collective_compute(
            "AllGather",
            mybir.AluOpType.bypass,
            replica_groups=[core_ids],
            ins=[cc_in.opt()],
            outs=[cc_out.opt()],
        )
        sg8 = wpool.tile([D + 1, N_CORES, K], BF16, tag="sg8")
        ccf = cc_out[0, 0, 0]  # base AP for offset/tensor
        nc.gpsimd.dma_start(
            out=sg8,
            in_=bass.AP(
                tensor=ccf.tensor, offset=ccf.offset,
                ap=[[K, D + 1], [(D + 1) * K, N_CORES], [1, K]],
            ),
        )
        with nc.allow_low_precision(reason="bf16 cross-core segment sums"):
            sg4 = wpool.tile([D + 1, 4, K], BF16, tag="sg4")
            nc.vector.tensor_add(sg4, sg8[:, :4, :], sg8[:, 4:, :])
            sg2t = wpool.tile([D + 1, 2, K], BF16, tag="sg2t")
            nc.vector.tensor_add(sg2t, sg4[:, :2, :], sg4[:, 2:, :])
        sg = wpool.tile([D + 1, K], F32, tag="sg2")
        nc.vector.tensor_tensor(
            sg, sg2t[:, 0, :], sg2t[:, 1, :], mybir.AluOpType.add
        )

        # ---------- ht builds (no AR dependency) ----------
        ht_tiles = []
        for oc in range(n_oc):
            l2, tn = l2_tiles[oc]
            ht = htpool.tile([P, TPAIR * P], BF16, tag="ht")
            nc.vector.tensor_single_scalar(
                ht[:, : tn * P], l2[:, : tn * P], iotacol2,
                mybir.AluOpType.is_equal,
            )
            ht_tiles.append(ht)

        # ---------- stats (Act/Pool/PE only; DVE stays on one-hot work) ----
        psW = psS.tile([K, D + 1], F32, tag="small")
        nc.tensor.transpose(psW, sg, id33)
        W = wpool.tile([K, D + 1], F32, tag="w")  # [S_k | c_k]
        nc.scalar.copy(out=W, in_=psW)
        safec = wpool.tile([K, 1], F32, tag="safec")
        nc.gpsimd.tensor_scalar(
            safec, W[:, D : D + 1], 256.0, 1.0,
            mybir.AluOpType.mult, mybir.AluOpType.max,
        )
        invc = wpool.tile([K, 1], F32, tag="invc")
        nc.vector.reciprocal(invc, safec)
        svp = wpool.tile([K, 1], F32, tag="svp")  # sqrt(1/c)
        nc.scalar.activation(
            out=svp, in_=invc, func=mybir.ActivationFunctionType.Sqrt
        )
        mu = wpool.tile([K, D], F32, tag="mu")
        nc.gpsimd.tensor_mul(mu, W[:, :D], invc.to_broadcast((K, D)))
        # table2 [128, 33] bf16 = [eps - mu | sqrt(1/c) - 1], rows replicated
        table2 = singles.tile([P, D + 1], BF16)
        nc.scalar.activation(
            out=table2[:K, :D], in_=mu,
            func=mybir.ActivationFunctionType.Copy, bias=EPS, scale=-1.0,
        )
        nc.scalar.activation(
            out=table2[:K, D : D + 1], in_=svp,
            func=mybir.ActivationFunctionType.Copy, bias=-1.0 / 256.0,
        )
        psTF = psS.tile([P, D + 1], F32, tag="small")
        nc.tensor.matmul(psTF, idrep, table2[:K, :])
        table2f = singles.tile([P, D + 1], BF16)
        nc.scalar.copy(out=table2f, in_=psTF)

        # ---------- inter + reg losses (Act/Pool/PE) ----------
        mup = wpool.tile([K, D], F32, tag="mup")  # mu + eps
        nc.scalar.activation(
            out=mup, in_=mu, func=mybir.ActivationFunctionType.Copy, bias=EPS
        )
        qsc = wpool.tile([K, D], F32, tag="qsc")
        nc.gpsimd.tensor_mul(qsc, mu, mu)
        q = wpool.tile([K, 1], F32, tag="q")  # ||mu||^2
        nc.vector.tensor_reduce(
            out=q, in_=qsc, axis=mybir.AxisListType.X, op=mybir.AluOpType.add
        )
        qpsc = wpool.tile([K, D], F32, tag="qpsc")
        nc.gpsimd.tensor_mul(qpsc, mup, mup)
        qp = wpool.tile([K, 1], F32, tag="qp")  # ||mu + eps||^2
        nc.vector.tensor_reduce(
            out=qp, in_=qpsc, axis=mybir.AxisListType.X, op=mybir.AluOpType.add
        )
        # pd2[a,b] = qp_a - 2*mup_a.mu_b + q_b via [ -2*mup | qp | 1 ] x [ mu | 1 | q ]
        ab = wpool.tile([K, D + 2], F32, tag="ab")
        nc.scalar.mul(out=ab[:, :D], in_=mup, mul=-2.0)
        nc.scalar.copy(out=ab[:, D : D + 1], in_=qp)
        nc.gpsimd.memset(ab[:, D + 1 : D + 2], 1.0)
        bb = wpool.tile([K, D + 2], F32, tag="bb")
        nc.scalar.copy(out=bb[:, :D], in_=mu)
        nc.gpsimd.memset(bb[:, D : D + 1], 1.0)
        nc.scalar.copy(out=bb[:, D + 1 : D + 2], in_=q)
        psT = psS.tile([D + 2, K], F32, tag="small")
        nc.tensor.transpose(psT, ab, id64)
        atp = wpool.tile([D + 2, K], F32, tag="atp")
        nc.scalar.copy(out=atp, in_=psT)
        psT2 = psS.tile([D + 2, K], F32, tag="small")
        nc.tensor.transpose(psT2, bb, id64)
        btp = wpool.tile([D + 2, K], F32, tag="btp")
        nc.scalar.copy(out=btp, in_=psT2)
        psPD = psS.tile([K, K], F32, tag="small")
        nc.tensor.matmul(psPD, atp, btp)
        pdc = wpool.tile([K, K], F32, tag="pdc")
        nc.vector.tensor_scalar_max(pdc, psPD, 0.0)
        pdist = wpool.tile([K, K], F32, tag="pdist")
        nc.scalar.activation(
            out=pdist, in_=pdc, func=mybir.ActivationFunctionType.Sqrt
        )
        hingeI = wpool.tile([K, K], F32, tag="hingeI")
        nc.scalar.activation(
            out=hingeI, in_=pdist, func=mybir.ActivationFunctionType.Relu,
            bias=float(INTER_MARGIN2), scale=-1.0,
        )
        hm = wpool.tile([K, K], F32, tag="hm")
        nc.gpsimd.tensor_mul(hm, hingeI, eyeneg)
        hm2 = wpool.tile([K, K], F32, tag="hm2")
        nc.gpsimd.tensor_mul(hm2, hm, hm)
        interp = wpool.tile([K, 1], F32, tag="interp")
        nc.vector.tensor_reduce(
            out=interp, in_=hm2, axis=mybir.AxisListType.X,
            op=mybir.AluOpType.add,
        )
        sqp = wpool.tile([K, 1], F32, tag="sqp")  # ||mu + eps||
        nc.scalar.activation(
            out=sqp, in_=qp, func=mybir.ActivationFunctionType.Sqrt
        )
        cat2 = wpool.tile([K, 2], F32, tag="cat2")
        nc.scalar.copy(out=cat2[:, 0:1], in_=interp)
        nc.scalar.copy(out=cat2[:, 1:2], in_=sqp)
        ones64 = singles.tile([K, 1], F32)
        nc.gpsimd.memset(ones64, 1.0)
        psIR = psS.tile([1, 2], F32, tag="small")
        nc.tensor.matmul(psIR, ones64, cat2)
        ir = wpool.tile([1, 2], F32, tag="ir")  # [inter_sum, reg_sum]
        nc.scalar.copy(out=ir, in_=psIR)

        # ---------- pass 2: gather + diff in PSUM, square, fold-reduce ------
        fin_oc = sorted(set(
            [n_oc - 1] + [max(0, (n_oc * (q + 1)) // 4 - 1) for q in range(3)]
        ))
        fin_base = []
        prev = 0
        for oc_ in fin_oc:
            fin_base.append(prev)
            pc_ = min(TPAIR, npair - oc_ * TPAIR)
            prev = min(oc_ * TPAIR * 2 + pc_ * 2, tpc)
        for oc in range(n_oc):
            ht = ht_tiles[oc]
            pc = min(TPAIR, npair - oc * TPAIR)
            cbase = oc * TPAIR * 2        # first global tile of this chunk
            ctn = min(pc * 2, tpc - cbase)  # tiles in this chunk
            # one sq tile per ht chunk (up to 56 tiles), 4 PSUM groups
            sq = sqpool.tile([P, TPAIR * 2, D + 1], BF16, tag="sq")
            for g in range(math.ceil(pc / (JMG // 2))):
                p0 = g * (JMG // 2)
                pn = min(JMG // 2, pc - p0)
                jbase = (oc * TPAIR + p0) * 2  # first global tile of group
                nt = min(pn * 2, tpc - jbase)
                psD = psMg.tile([P, JMG, D + 1], F32, tag="psd")
                for lp in range(pn):
                    for half in range(2):
                        t = lp * 2 + half
                        if t >= nt:
                            break
                        colp = p0 + lp
                        nc.tensor.matmul(
                            psD[:, t, :],
                            ht[64 * half : 64 * (half + 1),
                               colp * P : (colp + 1) * P],
                            table2f[64 * half : 64 * (half + 1), :],
                            start=True, stop=False,
                        )
                        nc.tensor.matmul(
                            psD[:, t, :], id128,
                            xe[:, cbase + p0 * 2 + t, :],
                            start=False, stop=True,
                        )
                t0 = p0 * 2
                if dve_sq_every and (oc * 4 + g) % dve_sq_every == dve_sq_every - 1:
                    # DVE square: PSUM copy then bf16 self-mult (one PSUM
                    # input per instruction as required by hardware)
                    cpy = sqpool.tile([P, JMG, D + 1], BF16, tag="cpy")
                    nc.vector.tensor_scalar_add(
                        cpy[:, :nt, :], psD[:, :nt, :], 0.0
                    )
                    nc.vector.tensor_mul(
                        sq[:, t0 : t0 + nt, :], cpy[:, :nt, :], cpy[:, :nt, :]
                    )
                else:
                    nc.scalar.activation(
                        out=sq[:, t0 : t0 + nt, :], in_=psD[:, :nt, :],
                        func=mybir.ActivationFunctionType.Square,
                    )
            # bf16 fold-tree reduce over D (2x DVE mode); f1 of every other
            # chunk goes to Pool to offload DVE
            with nc.allow_low_precision(reason="bf16 partial sums of d2"):
                f1 = sqpool.tile([P, TPAIR * 2, 16], BF16, tag="f1")
                f1eng = nc.vector if (f1_dve_every and oc % f1_dve_every == f1_dve_every - 1) else nc.gpsimd
                f1eng.tensor_add(
                    f1[:, :ctn, :], sq[:, :ctn, 0:16], sq[:, :ctn, 16:32]
                )
                f2 = sqpool.tile([P, TPAIR * 2, 8], BF16, tag="f2")
                nc.vector.tensor_add(
                    f2[:, :ctn, :], f1[:, :ctn, 0:8], f1[:, :ctn, 8:16]
                )
                f3 = sqpool.tile([P, TPAIR * 2, 4], BF16, tag="f3")
                nc.vector.tensor_add(
                    f3[:, :ctn, :], f2[:, :ctn, 0:4], f2[:, :ctn, 4:8]
                )
                f4 = sqpool.tile([P, TPAIR * 2, 2], BF16, tag="f4")
                nc.vector.tensor_add(
                    f4[:, :ctn, :], f3[:, :ctn, 0:2], f3[:, :ctn, 2:4]
                )
            nc.vector.tensor_tensor(
                d2all[:, cbase : cbase + ctn],
                f4[:, :ctn, 0], f4[:, :ctn, 1], mybir.AluOpType.add,
            )
            nc.gpsimd.tensor_scalar_add(
                invc_all[:, cbase : cbase + ctn], sq[:, :ctn, D], 0.0
            )
            # quarter-granularity finals (keeps the serial tail short
            # without flooding Act with per-chunk overhead):
            # dist = sqrt(d2); h = relu(dist - 1.5); acc_q = sum h^2 * invc
            if oc in fin_oc:
                qi = fin_oc.index(oc)
                b0 = fin_base[qi]
                b1 = cbase + ctn
                dsl = d2all[:, b0:b1]
                nc.scalar.activation(
                    out=dsl, in_=dsl,
                    func=mybir.ActivationFunctionType.Sqrt,
                )
                nc.scalar.activation(
                    out=dsl, in_=dsl,
                    func=mybir.ActivationFunctionType.Relu, bias=margneg,
                )
                hsl = hh[:, b0:b1]
                nc.vector.tensor_mul(hsl, dsl, dsl)
                wsl = hhw[:, b0:b1]
                nc.vector.tensor_mul(wsl, hsl, invc_all[:, b0:b1])
                nc.vector.tensor_reduce(
                    out=rsacc[:, qi : qi + 1], in_=wsl,
                    axis=mybir.AxisListType.X, op=mybir.AluOpType.add,
                )

        # ---------- finals: reduce per-quarter partials ----------
        rowsum = singles.tile([P, 1], F32)
        nc.vector.tensor_reduce(
            out=rowsum, in_=rsacc[:, : len(fin_oc)],
            axis=mybir.AxisListType.X, op=mybir.AluOpType.add,
        )
        ones128 = singles.tile([P, 1], F32)
        nc.gpsimd.memset(ones128, 1.0)
        psL = psS.tile([1, 1], F32, tag="small")
        nc.tensor.matmul(psL, rowsum, ones128)
        tot = wpool.tile([1, 3], F32, tag="tot")
        nc.scalar.copy(out=tot[:, 0:1], in_=psL)
        nc.scalar.copy(out=tot[:, 1:3], in_=ir)
        nc.sync.dma_start(out=out_d, in_=tot[0:1, :])

    nc.compile()
    return nc


_NC_CACHE = {}


def _get_program(tpc):
    if tpc not in _NC_CACHE:
        _NC_CACHE[tpc] = build_program(tpc)
    return _NC_CACHE[tpc]


def kernel(features, labels, num_clusters):
    features = np.asarray(features)
    labels = np.asarray(labels)
    n_total = features.shape[0]
    n_core = n_total // N_CORES
    tpc = math.ceil(n_core / P)
    nc = _get_program(tpc)
    in_maps = _host_prep(features, labels, tpc)
    res = run_bass_kernel_spmd(nc, in_maps, list(range(N_CORES)))
    intra_sum = sum(float(res.results[c]["out"][0]) for c in range(N_CORES))
    inter_sum = float(res.results[0]["out"][1])
    reg_sum = float(res.results[0]["out"][2])
    total = (
        intra_sum / K
        + inter_sum / (K * (K - 1))
        + 0.001 * reg_sum / K
    )
    return np.float32(total)
